# revision 1
# baseline (speedup 1.0000x reference)
"""Bass/Trainium2 kernel for nn_HNO_37065567764989 (self-contained).

Strategy (8 NeuronCores, SPMD):
- The 128x524288 branch matvec b = Wb@a is column-sharded 8 ways. Each core
  streams its 32MB shard as bf16 hi/lo pairs (exact-ish split, ~4e-6 rel) with
  the [a_hi|a_lo] pair as a K=128/M=2 stationary and [W_hi|W_lo] as one N=256
  moving operand, accumulating all four cross terms in PSUM. A 512B AllReduce
  combines the per-core partials.
- The Nx=32768 evaluation points are sharded 8 ways (4096/core). The trunk
  chains (b-independent) overlap the Wb DMA; the EnergyNet phase runs after
  the AllReduce, chunk-stacked two 64-wide halves into 128 partitions.
- Output is -d/dx F_y + d^2/dx^2 F_z computed via forward derivative chains;
  all constant factors fold into matmul stationaries, sums fold into PSUM
  accumulation.
"""
import sys

for _p in ("/opt/trn_rl_repo",):
    if _p not in sys.path:
        sys.path.insert(0, _p)

import numpy as np
import ml_dtypes

MP1, NX, P, HT, HE = 524288, 32768, 128, 128, 64
NCORES = 8
KSH = MP1 // NCORES       # 65536
NCHUNK = 32
JT = KSH // NCHUNK // 128  # 16 k-tiles per chunk
NPTS = NX // NCORES        # 4096 points per core
FD = 512
NTRUNK = NPTS // FD        # 8 trunk tiles
NEN = NPTS // 2 // FD      # 4 energy tiles (chunk-stacked)

_CACHE = {}


def _build(mmdt_name):
    import concourse.bacc as bacc
    import concourse.mybir as mybir
    from concourse import tile

    f32 = mybir.dt.float32
    bf16 = mybir.dt.bfloat16
    MMDT = getattr(mybir.dt, mmdt_name)
    AF = mybir.ActivationFunctionType
    ALU = mybir.AluOpType

    nc = bacc.Bacc("TRN2", target_bir_lowering=False, debug=False,
                   num_devices=NCORES)

    w_d = nc.dram_tensor("w", [NCHUNK, 2, 128, JT * 128], bf16, kind="ExternalInput")
    a_d = nc.dram_tensor("a2", [128, KSH // 128, 2], bf16, kind="ExternalInput")
    x2_d = nc.dram_tensor("x2", [2, NPTS], MMDT, kind="ExternalInput")
    sm = {}
    for name, shape, dt in [
        ("w10_2", [2, 128], MMDT), ("c1b", [128, 1], f32), ("bt2b", [128, 1], f32),
        ("wt2t", [128, 128], MMDT), ("w2at", [128, 128], MMDT),
        ("w2bt", [128, 128], MMDT), ("w2ct", [128, 128], MMDT),
        ("wt3", [128, 128], f32), ("bt3c", [128, 1], f32),
        ("e0", [128, 128], MMDT), ("e0m2", [128, 128], MMDT),
        ("eq", [128, 128], MMDT), ("eqm2", [128, 128], MMDT),
        ("eq6", [128, 128], MMDT), ("ep", [128, 128], MMDT),
        ("epm2", [128, 128], MMDT),
        ("be1b2", [128, 1], f32), ("be2b2", [128, 1], f32),
        ("lmat", [4, 768], MMDT), ("v6", [128, 12], MMDT),
    ]:
        sm[name] = nc.dram_tensor(name, shape, dt, kind="ExternalInput")
    out_d = nc.dram_tensor("out", [2, NPTS // 2], f32, kind="ExternalOutput")
    cc_in = nc.dram_tensor("cc_in", [128, 1], f32)
    cc_out = nc.dram_tensor("cc_out", [128, 1], f32, addr_space="Shared")

    def TT(eng, out, a, b, op=ALU.mult):
        eng.tensor_tensor(out, a, b, op)

    with tile.TileContext(nc) as tc:
        with (
            tc.tile_pool(name="smp", bufs=1) as smp,
            tc.tile_pool(name="persist", bufs=1) as persist,
            tc.tile_pool(name="wpool", bufs=3) as wpool,
            tc.tile_pool(name="scr", bufs=1) as scr,
            tc.tile_pool(name="u8p", bufs=1) as u8p,
        ):
            # small weights to SBUF
            smt = {}
            for name, h in sm.items():
                t = smp.tile(list(h.shape), h.dtype, name=f"sb_{name}")
                nc.sync.dma_start(t[:], h.ap())
                smt[name] = t
            a2 = smp.tile([128, KSH // 128, 2], bf16, name="a2t")
            nc.sync.dma_start(a2[:], a_d.ap())
            ones2 = smp.tile([2, 1], f32, name="ones2")
            nc.vector.memset(ones2[:], 1.0)

            persist_t = {}
            with tc.tile_pool(name="ps8", bufs=1, space="PSUM") as ps8:
                mvps = tkps = exps = enps = ps8
                # ---- matvec: stream Wb shard ----
                b2_ps = mvps.tile([2, 256], f32, tag="pG", name="b2ps")
                for i in range(NCHUNK):
                    wch = wpool.tile([128, 2, JT * 128], bf16, tag="wch", name="wch")
                    nc.sync.dma_start(wch[:], w_d.ap()[i].rearrange("s kp x -> kp s x"))
                    for j in range(JT):
                        jj = i * JT + j
                        nc.tensor.matmul(
                            b2_ps[:], a2[:, jj, :],
                            wch[:, :, j * 128:(j + 1) * 128],
                            start=(jj == 0), stop=(jj == NCHUNK * JT - 1),
                        )
                b2_sb = smp.tile([2, 256], f32, name="b2sb")
                nc.scalar.copy(b2_sb[:], b2_ps[:])
                bcol_ps = mvps.tile([128, 1], f32, tag="pH", name="bcolps")
                nc.tensor.matmul(bcol_ps[:], b2_sb[:, 0:128], ones2[:], start=True, stop=False)
                nc.tensor.matmul(bcol_ps[:], b2_sb[:, 128:256], ones2[:], start=False, stop=True)
                b_loc = smp.tile([128, 1], f32, name="bloc")
                nc.scalar.copy(b_loc[:], bcol_ps[:])
                nc.sync.dma_start(cc_in.ap(), b_loc[:])
                nc.gpsimd.collective_compute(
                    "AllReduce", ALU.add,
                    replica_groups=[list(range(NCORES))],
                    ins=[cc_in.ap()], outs=[cc_out.ap()],
                )
                b_ar = smp.tile([128, 1], f32, name="bar")
                nc.sync.dma_start(b_ar[:], cc_out.ap())

                # ---- trunk (b-independent), overlaps the matvec DMA ----
                for f in range(NTRUNK):
                    cs = slice(f * FD, (f + 1) * FD)
                    xt = scr.tile([2, FD], MMDT, tag="xt", name="xt", bufs=2)
                    nc.sync.dma_start(xt[:], x2_d.ap()[:, cs])
                    z1 = tkps.tile([128, FD], f32, tag="pA", name="z1")
                    nc.tensor.matmul(z1[:], smt["w10_2"][:], xt[:], start=True, stop=True)
                    t1 = scr.tile([128, FD], MMDT, tag="t1", name="t1")
                    nc.scalar.activation(t1[:], z1[:], AF.Tanh, bias=smt["c1b"][:])
                    t1f = t1[:].bitcast(f32)
                    s1 = scr.tile([128, FD], f32, tag="s1", name="s1")
                    nc.scalar.square(s1[:], t1f)
                    tp1 = scr.tile([128, FD], MMDT, tag="tp1", name="tp1")
                    nc.vector.tensor_scalar(tp1[:], s1[:], -1.0, 1.0, ALU.mult, ALU.add)
                    tp1f = tp1[:].bitcast(f32)
                    g2m = scr.tile([128, FD], MMDT, tag="g2m", name="g2m")
                    TT(nc.vector, g2m[:], t1f, tp1f)
                    g3m = scr.tile([128, FD], MMDT, tag="g3m", name="g3m")
                    nc.vector.scalar_tensor_tensor(
                        g3m[:], s1[:], 1.0 / 3.0, tp1f, ALU.subtract, ALU.mult)
                    z2 = tkps.tile([128, FD], f32, tag="pB", name="z2")
                    nc.tensor.matmul(z2[:], smt["wt2t"][:], t1[:], start=True, stop=True)
                    A = tkps.tile([128, FD], f32, tag="pC", name="A")
                    nc.tensor.matmul(A[:], smt["w2at"][:], tp1[:], start=True, stop=True)
                    B = tkps.tile([128, FD], f32, tag="pD", name="B")
                    nc.tensor.matmul(B[:], smt["w2bt"][:], g2m[:], start=True, stop=True)
                    C = tkps.tile([128, FD], f32, tag="pE", name="C")
                    nc.tensor.matmul(C[:], smt["w2ct"][:], g3m[:], start=True, stop=True)

                    t2 = persist.tile([128, FD], MMDT, tag=f"t2_{f}", name=f"t2_{f}")
                    nc.scalar.activation(t2[:], z2[:], AF.Tanh, bias=smt["bt2b"][:])
                    t2f = t2[:].bitcast(f32)
                    s2 = scr.tile([128, FD], f32, tag="s2", name="s2")
                    nc.scalar.square(s2[:], t2f)
                    tp2 = scr.tile([128, FD], f32, tag="tp2", name="tp2")
                    nc.vector.tensor_scalar(tp2[:], s2[:], -1.0, 1.0, ALU.mult, ALU.add)
                    A2 = scr.tile([128, FD], f32, tag="A2", name="A2")
                    nc.scalar.square(A2[:], A[:])
                    P1 = persist.tile([128, FD], MMDT, tag=f"P1_{f}", name=f"P1_{f}")
                    TT(nc.vector, P1[:], tp2[:], A[:])
                    M4 = scr.tile([128, FD], f32, tag="M4", name="M4")
                    TT(nc.vector, M4[:], tp2[:], A2[:])
                    M5 = scr.tile([128, FD], f32, tag="M5", name="M5")
                    TT(nc.gpsimd, M5[:], t2f, M4[:])
                    M6 = scr.tile([128, FD], f32, tag="M6", name="M6")
                    TT(nc.vector, M6[:], tp2[:], B[:])
                    # uxxM = -2*M5 + M6
                    uxxM = persist.tile([128, FD], MMDT, tag=f"ux2_{f}", name=f"ux2_{f}")
                    nc.vector.scalar_tensor_tensor(
                        uxxM[:], M5[:], -2.0, M6[:], ALU.mult, ALU.add)
                    A3 = scr.tile([128, FD], f32, tag="A3", name="A3")
                    TT(nc.vector, A3[:], A2[:], A[:])
                    V = scr.tile([128, FD], f32, tag="V", name="V")
                    nc.vector.scalar_tensor_tensor(
                        V[:], s2[:], 1.0 / 3.0, tp2[:], ALU.subtract, ALU.mult)
                    M1 = scr.tile([128, FD], f32, tag="M1", name="M1")
                    TT(nc.gpsimd, M1[:], V[:], A3[:])
                    W1 = scr.tile([128, FD], f32, tag="W1", name="W1")
                    TT(nc.vector, W1[:], P1[:].bitcast(f32), B[:])
                    M2 = scr.tile([128, FD], f32, tag="M2", name="M2")
                    TT(nc.gpsimd, M2[:], t2f, W1[:])
                    M3 = scr.tile([128, FD], f32, tag="M3", name="M3")
                    TT(nc.vector, M3[:], tp2[:], C[:])
                    # uxxxM = 6*(M1 - M2) + M3
                    D1 = scr.tile([128, FD], f32, tag="D1", name="D1")
                    TT(nc.vector, D1[:], M1[:], M2[:], ALU.subtract)
                    uxxxM = persist.tile([128, FD], MMDT, tag=f"ux3_{f}", name=f"ux3_{f}")
                    nc.vector.scalar_tensor_tensor(
                        uxxxM[:], D1[:], 6.0, M3[:], ALU.mult, ALU.add)
                    persist_t[f] = (t2, P1, uxxM, uxxxM)

                # ---- post-AllReduce: c, d ----
                c_ps = exps.tile([128, 1], f32, tag="pA", name="cps")
                nc.tensor.matmul(c_ps[:], smt["wt3"][:], b_ar[:], start=True, stop=True)
                c_sb = smp.tile([128, 1], MMDT, name="csb")
                nc.scalar.copy(c_sb[:], c_ps[:])
                d_ps = exps.tile([1, 1], f32, tag="pB", name="dps")
                nc.tensor.matmul(d_ps[:], smt["bt3c"][:], b_ar[:], start=True, stop=True)
                d_sb = smp.tile([1, 1], f32, name="dsb")
                nc.scalar.copy(d_sb[:], d_ps[:])
                def do_extract(f):
                    t2, P1, uxxM, uxxxM = persist_t[f]
                    half = u8p.tile([4, FD], MMDT, tag=f"u8_{f}", name=f"u8_{f}")
                    tagmap = ["pC", "pD", "pE", "pF"]
                    for r, (mov, bias) in enumerate([
                        (t2, d_sb), (P1, None), (uxxM, None), (uxxxM, None),
                    ]):
                        ue = exps.tile([1, FD], f32, tag=tagmap[r], name=f"ue{r}")
                        nc.tensor.matmul(ue[:], c_sb[:], mov[:], start=True, stop=True)
                        ur = scr.tile([1, FD], MMDT, tag=f"ur{r}", name=f"ur{r}")
                        if bias is not None:
                            nc.scalar.activation(ur[:], ue[:], AF.Identity, bias=bias[:])
                        else:
                            nc.scalar.copy(ur[:], ue[:])
                        nc.sync.dma_start(half[r:r + 1, :], ur[:])
                    return half

                # ---- energy phase, extracts interleaved ----
                L = smt["lmat"]
                for e in range(NEN):
                    hA = do_extract(e)
                    hB = do_extract(e + NEN)
                    z1e = enps.tile([128, FD], f32, tag="pA", name="z1e")
                    z1p = enps.tile([128, FD], f32, tag="pB", name="z1p")
                    z1pp = enps.tile([128, FD], f32, tag="pC", name="z1pp")
                    for ps_t, li in ((z1e, 0), (z1p, 1), (z1pp, 2)):
                        nc.tensor.matmul(ps_t[:], L[:, li * 256:li * 256 + 128],
                                         hA[:], start=True, stop=False)
                        nc.tensor.matmul(ps_t[:], L[:, li * 256 + 128:(li + 1) * 256],
                                         hB[:], start=False, stop=True)
                    t1e = scr.tile([128, FD], MMDT, tag="t1", name="t1e")
                    nc.scalar.activation(t1e[:], z1e[:], AF.Tanh, bias=smt["be1b2"][:])
                    t1ef = t1e[:].bitcast(f32)
                    s1e = scr.tile([128, FD], f32, tag="s1", name="s1e")
                    nc.scalar.square(s1e[:], t1ef)
                    m = scr.tile([128, FD], MMDT, tag="tp1", name="m")
                    nc.vector.tensor_scalar(m[:], s1e[:], -1.0, 1.0, ALU.mult, ALU.add)
                    mf = m[:].bitcast(f32)
                    z1p2 = scr.tile([128, FD], f32, tag="g2m", name="z1p2")
                    nc.scalar.square(z1p2[:], z1p[:])
                    N1 = scr.tile([128, FD], f32, tag="g3m", name="N1")
                    TT(nc.gpsimd, N1[:], t1ef, mf)
                    a1p = scr.tile([128, FD], MMDT, tag="s2", name="a1p")
                    TT(nc.vector, a1p[:], mf, z1p[:])
                    N2 = scr.tile([128, FD], MMDT, tag="tp2", name="N2")
                    TT(nc.gpsimd, N2[:], N1[:], z1p2[:])
                    N3 = scr.tile([128, FD], MMDT, tag="A2", name="N3")
                    TT(nc.vector, N3[:], mf, z1pp[:])
                    mpc = scr.tile([128, FD], MMDT, tag="M4", name="mpc")
                    TT(nc.vector, mpc[:], N1[:], z1p[:])
                    O1 = scr.tile([128, FD], f32, tag="M5", name="O1")
                    nc.vector.scalar_tensor_tensor(
                        O1[:], s1e[:], 1.0 / 3.0, mf, ALU.subtract, ALU.mult)
                    O2 = scr.tile([128, FD], MMDT, tag="M6", name="O2")
                    TT(nc.gpsimd, O2[:], O1[:], z1p2[:])
                    O3 = scr.tile([128, FD], MMDT, tag="A3", name="O3")
                    TT(nc.vector, O3[:], N1[:], z1pp[:])

                    z2e = enps.tile([128, FD], f32, tag="pD", name="z2e")
                    nc.tensor.matmul(z2e[:], smt["e0"][:], t1e[:], start=True, stop=True)
                    t2e = scr.tile([128, FD], f32, tag="V", name="t2e")
                    nc.scalar.activation(t2e[:], z2e[:], AF.Tanh, bias=smt["be2b2"][:])
                    z2ep = enps.tile([128, FD], f32, tag="pE", name="z2ep")
                    nc.tensor.matmul(z2ep[:], smt["e0"][:], a1p[:], start=True, stop=True)
                    z2epp = enps.tile([128, FD], f32, tag="pF", name="z2epp")
                    nc.tensor.matmul(z2epp[:], smt["e0m2"][:], N2[:], start=True, stop=False)
                    nc.tensor.matmul(z2epp[:], smt["e0"][:], N3[:], start=False, stop=True)
                    Dz = enps.tile([128, FD], f32, tag="pG", name="Dz")
                    nc.tensor.matmul(Dz[:], smt["eq"][:], m[:], start=True, stop=True)
                    Dy = enps.tile([128, FD], f32, tag="pH", name="Dy")
                    nc.tensor.matmul(Dy[:], smt["ep"][:], m[:], start=True, stop=True)
                    Dzp = enps.tile([128, FD], f32, tag="pA", name="Dzp")
                    nc.tensor.matmul(Dzp[:], smt["eqm2"][:], mpc[:], start=True, stop=True)
                    Dyp = enps.tile([128, FD], f32, tag="pB", name="Dyp")
                    nc.tensor.matmul(Dyp[:], smt["epm2"][:], mpc[:], start=True, stop=True)
                    Dzpp = enps.tile([128, FD], f32, tag="pC", name="Dzpp")
                    nc.tensor.matmul(Dzpp[:], smt["eq6"][:], O2[:], start=True, stop=False)
                    nc.tensor.matmul(Dzpp[:], smt["eqm2"][:], O3[:], start=False, stop=True)

                    s2e = scr.tile([128, FD], f32, tag="M1", name="s2e")
                    nc.scalar.square(s2e[:], t2e[:])
                    w = scr.tile([128, FD], f32, tag="AB", name="w")
                    nc.vector.tensor_scalar(w[:], s2e[:], -1.0, 1.0, ALU.mult, ALU.add)
                    z2ep2 = scr.tile([128, FD], f32, tag="W1", name="z2ep2")
                    nc.scalar.square(z2ep2[:], z2ep[:])
                    Q1 = scr.tile([128, FD], f32, tag="M2", name="Q1")
                    TT(nc.gpsimd, Q1[:], t2e[:], w[:])
                    wpc = scr.tile([128, FD], f32, tag="M3", name="wpc")
                    TT(nc.vector, wpc[:], Q1[:], z2ep[:])
                    R1 = scr.tile([128, FD], f32, tag="D1", name="R1")
                    nc.vector.scalar_tensor_tensor(
                        R1[:], s2e[:], 1.0 / 3.0, w[:], ALU.subtract, ALU.mult)
                    R2 = scr.tile([128, FD], f32, tag="ur0", name="R2")
                    TT(nc.gpsimd, R2[:], R1[:], z2ep2[:])
                    R3 = scr.tile([128, FD], f32, tag="ur1", name="R3")
                    TT(nc.vector, R3[:], Q1[:], z2epp[:])

                    vps = enps.tile([2, FD], f32, tag="pD", name="vps")
                    fsrc = [(R2, Dz), (R3, Dz), (wpc, Dzp), (wpc, Dy), (w, Dzpp), (w, Dyp)]
                    for i, (x1, x2_) in enumerate(fsrc):
                        Fi = scr.tile([128, FD], MMDT, tag=["ur2", "ur3", "fm2"][i % 3], name=f"f{i}")
                        TT(nc.vector, Fi[:], x1[:], x2_[:])
                        nc.tensor.matmul(vps[:], smt["v6"][:, 2 * i:2 * i + 2], Fi[:],
                                         start=(i == 0), stop=(i == 5))
                    ot = scr.tile([2, FD], f32, tag="ot", name="ot")
                    nc.scalar.copy(ot[:], vps[:])
                    nc.sync.dma_start(
                        out_d.ap()[:, e * FD:(e + 1) * FD], ot[:])

    nc.compile()
    return nc


def _get_nc(mmdt_name):
    if mmdt_name not in _CACHE:
        _CACHE[mmdt_name] = _build(mmdt_name)
    return _CACHE[mmdt_name]


MMDT_NAME = "float32"


def kernel(**inputs):
    import concourse.bass_utils as bass_utils

    f = lambda k: np.asarray(inputs[k], np.float32)
    a, x, t = f("a"), f("x"), np.float32(inputs["t"])
    Wb, Wt1, bt1, Wt2, bt2 = f("Wb"), f("Wt1"), f("bt1"), f("Wt2"), f("bt2")
    Wt3, bt3, We1, be1, We2, be2, We3 = (
        f("Wt3"), f("bt3"), f("We1"), f("be1"), f("We2"), f("be2"), f("We3"))
    bb, be3 = f("bb"), f("be3")

    w1 = Wt1[:, 0]
    c1b = (Wt1[:, 1] * t + bt1)[:, None]
    p, q, v = We1[:, 0], We1[:, 1], We3[0]
    blk = lambda M: np.block([[M, np.zeros_like(M)], [np.zeros_like(M), M]])
    We2T = We2.T
    lmat = np.zeros((4, 768), np.float32)
    for li, pat in enumerate([(p, q, None, None), (None, p, q, None), (None, None, p, q)]):
        for r, vec in enumerate(pat):
            if vec is not None:
                lmat[r, li * 256:li * 256 + 64] = 0  # placeholder, set below
    # build lmat properly: cols 0-127 chunk A, 128-255 chunk B per member
    lmat = np.zeros((4, 768), np.float32)
    for li in range(3):
        rows = [(li + 0, p), (li + 1, q)]
        for r, vec in rows:
            lmat[r, li * 256:li * 256 + 64] = vec
            lmat[r, li * 256 + 128 + 64:li * 256 + 256] = vec
    v6 = np.zeros((128, 12), np.float32)
    for i, coef in enumerate([6.0, -2.0, -4.0, 2.0, 1.0, -1.0]):
        v6[0:64, 2 * i] = coef * v
        v6[64:128, 2 * i + 1] = coef * v

    smalls = {
        "w10_2": np.stack([w1, np.zeros_like(w1)]),
        "c1b": c1b, "bt2b": bt2[:, None],
        "wt2t": np.ascontiguousarray(Wt2.T),
        "w2at": np.ascontiguousarray(Wt2.T) * w1[:, None],
        "w2bt": np.ascontiguousarray(Wt2.T) * (-2.0 * w1 ** 2)[:, None],
        "w2ct": np.ascontiguousarray(Wt2.T) * (6.0 * w1 ** 3)[:, None],
        "wt3": Wt3, "bt3c": bt3[:, None],
        "e0": blk(We2T), "e0m2": blk(-2.0 * We2T),
        "eq": blk(We2T * q[:, None]), "eqm2": blk(-2.0 * We2T * q[:, None]),
        "eq6": blk(6.0 * We2T * q[:, None]), "ep": blk(We2T * p[:, None]),
        "epm2": blk(-2.0 * We2T * p[:, None]),
        "be1b2": np.concatenate([be1, be1])[:, None],
        "be2b2": np.concatenate([be2, be2])[:, None],
        "lmat": lmat, "v6": v6,
    }
    smalls = {k: np.ascontiguousarray(val, np.float32) for k, val in smalls.items()}

    in_maps = []
    for c in range(NCORES):
        blk_w = Wb[:, c * KSH:(c + 1) * KSH]
        tr = blk_w.T.reshape(NCHUNK, JT, 128, 128).transpose(0, 2, 1, 3)
        tr = tr.reshape(NCHUNK, 128, JT * 128)
        hi = tr.astype(ml_dtypes.bfloat16)
        lo = (tr - hi.astype(np.float32)).astype(ml_dtypes.bfloat16)
        wsh = np.ascontiguousarray(np.stack([hi, lo], axis=1))
        ash = a[c * KSH:(c + 1) * KSH].reshape(KSH // 128, 128).T
        ahi = ash.astype(ml_dtypes.bfloat16)
        alo = (ash - ahi.astype(np.float32)).astype(ml_dtypes.bfloat16)
        a2 = np.ascontiguousarray(np.stack([ahi, alo], axis=2))
        xs = x[c * NPTS:(c + 1) * NPTS]
        x2 = np.ascontiguousarray(np.stack([xs, np.zeros_like(xs)]))
        im = {"w": wsh, "a2": a2, "x2": x2}
        im.update(smalls)
        in_maps.append(im)

    global _last_in_maps
    _last_in_maps = in_maps
    nc = _get_nc(MMDT_NAME)
    res = bass_utils.run_bass_kernel_spmd(nc, in_maps, core_ids=list(range(NCORES)))
    outs = []
    for c in range(NCORES):
        o = res.results[c]["out"]          # [2, NPTS//2]
        outs.append(o.reshape(-1))
    return np.concatenate(outs).astype(np.float32)



# revision 19
# speedup vs baseline: 1.3577x; 1.3577x over previous
"""Bass/Trainium2 kernel for nn_HNO_37065567764989 (self-contained).

Strategy (8 NeuronCores, SPMD):
- Branch matvec b = Wb@a column-sharded 8 ways. Each core streams its 16MB
  shard as fp16 (W scaled by 2^10 to stay normal; a carried as an fp16 hi/lo
  stationary pair, M=8 batched over 4 k-tiles per matmul). 512B AllReduce
  combines partials.
- Nx=32768 points sharded 8 ways (4096/core). Trunk layer-1 overlaps the Wb
  stream; layer-2 (z2/A/B/C with fp16 hi/lo pair stationaries+movings) fills
  the AllReduce latency; the EnergyNet phase runs after, with u/u_x/u_xx/u_xxx
  extracted via an on-device c=Wt3^T b hi/lo pair stationary and first-layer
  preactivations built from stacked hi/lo row movings (2 matmuls each).
- Critical values flow at >=2^-24 effective precision (fp16 hi/lo pairs);
  low-sensitivity operands are single fp16. All matmuls run at 1 cy/row.
"""
import sys

for _p in ("/opt/trn_rl_repo",):
    if _p not in sys.path:
        sys.path.insert(0, _p)

import numpy as np

MP1, NX, P, HT, HE = 524288, 32768, 128, 128, 64
NCORES = 8
KSH = MP1 // NCORES        # 65536 contraction elems per core
NKT = KSH // 128           # 512 k-tiles
NCHUNK = 32
KTC = NKT // NCHUNK        # 16 k-tiles per chunk
NPTS = NX // NCORES        # 4096 points per core
FD = 512
NTRUNK = NPTS // FD        # 8 trunk tiles
NEN = NTRUNK // 2          # 4 energy tiles (two halves stacked)

_CACHE = {}


def _build():
    import concourse.bacc as bacc
    import concourse.mybir as mybir
    from concourse import tile

    f32 = mybir.dt.float32
    f16 = mybir.dt.float16
    AF = mybir.ActivationFunctionType
    ALU = mybir.AluOpType

    nc = bacc.Bacc("TRN2", target_bir_lowering=False, debug=False,
                   num_devices=NCORES)

    w_d = nc.dram_tensor("w", [NCHUNK, 128, KTC * 128], f16, kind="ExternalInput")
    a_d = nc.dram_tensor("a2", [128, NKT, 2], f16, kind="ExternalInput")
    x_d = nc.dram_tensor("x4", [4, NPTS], f16, kind="ExternalInput")
    sm = {}
    for name, shape, dt in [
        ("w11", [4, 128], f16), ("c1b", [128, 1], f32), ("bt2b", [128, 1], f32),
        ("wt2h", [128, 128], f16), ("wt2l", [128, 128], f16),
        ("w2ah", [128, 128], f16), ("w2al", [128, 128], f16),
        ("w2bh", [128, 128], f16), ("w2bl", [128, 128], f16),
        ("w2ch", [128, 128], f16), ("w2cl", [128, 128], f16),
        ("wt3h", [128, 128], f16), ("wt3l", [128, 128], f16),
        ("SH", [8, 128], f16), ("SL", [8, 128], f16),
        ("e0", [128, 128], f16), ("eq", [128, 128], f16), ("ep", [128, 128], f16),
        ("v6", [128, 6], f16),
        ("be1b2", [128, 1], f32), ("be2b2", [128, 1], f32),
        ("sel4m", [8, 4], f32),
    ]:
        sm[name] = nc.dram_tensor(name, shape, dt, kind="ExternalInput")
    out_d = nc.dram_tensor("out", [2, NPTS // 2], f32, kind="ExternalOutput")
    cc_in = nc.dram_tensor("cc_in", [128, 1], f32)
    cc_out = nc.dram_tensor("cc_out", [128, 1], f32, addr_space="Shared")

    def TT(eng, out, i0, i1, op=ALU.mult):
        eng.tensor_tensor(out, i0, i1, op)

    with tile.TileContext(nc) as tc:
        with (
            tc.tile_pool(name="smp", bufs=1) as smp,
            tc.tile_pool(name="persist", bufs=1) as persist,
            tc.tile_pool(name="wpool", bufs=3) as wpool,
            tc.tile_pool(name="scr", bufs=1) as scr,
            tc.tile_pool(name="ps8", bufs=1, space="PSUM") as ps,
        ):
            smt = {}
            for name, h in sm.items():
                t = smp.tile(list(h.shape), h.dtype, name=f"sb_{name}")
                nc.sync.dma_start(t[:], h.ap())
                smt[name] = t
            x4 = smp.tile([4, NPTS], f16, name="x4t")
            nc.sync.dma_start(x4[:], x_d.ap())
            a2 = smp.tile([128, NKT, 2], f16, name="a2t")
            nc.sync.dma_start(a2[:], a_d.ap())
            ones11 = smp.tile([1, 1], f32, name="ones11")
            nc.vector.memset(ones11[:], 1.0)

            # ---- trunk layer-1 z1 matmuls (before the matvec chain) ----
            z1tags = ["pT0", "pT1", "pB", "pC"]
            z1ps = []
            for f in range(NTRUNK):
                cs = slice(f * FD, (f + 1) * FD)
                z1 = ps.tile([128, FD], f32, tag=z1tags[f % 4], name=f"z1_{f}")
                nc.tensor.matmul(z1[:], smt["w11"][:], x4[:, cs], start=True, stop=True)
                z1ps.append(z1)

            # ---- matvec: stream W shard ----
            l1 = {}
            b8 = ps.tile([8, FD], f32, tag="pMV", name="b8")
            for i in range(NCHUNK):
                wch = wpool.tile([128, KTC * 128], f16, tag="wch", name="wch")
                nc.sync.dma_start(wch[:], w_d.ap()[i])
                for g in range(4):
                    nc.tensor.matmul(
                        b8[:], a2[:, i * KTC + 4 * g:i * KTC + 4 * (g + 1), :],
                        wch[:, g * 512:(g + 1) * 512],
                        start=(i == 0 and g == 0),
                        stop=(i == NCHUNK - 1 and g == 3),
                    )
                # trunk layer-1 elementwise rides under the DMA stream
                if i < NTRUNK:
                    f = i
                    z1 = z1ps[f]
                    t1f = scr.tile([128, FD], f32, tag=f"t1f{f % 2}", name=f"t1f_{f}")
                    nc.scalar.activation(t1f[:], z1[:], AF.Tanh, bias=smt["c1b"][:])
                    t1h = persist.tile([128, FD], f16, tag=f"t1h_{f}", name=f"t1h_{f}")
                    nc.scalar.copy(t1h[:], t1f[:])
                    t1l = persist.tile([128, FD], f16, tag=f"t1l_{f}", name=f"t1l_{f}")
                    TT(nc.vector, t1l[:], t1f[:], t1h[:], ALU.subtract)
                    s1 = scr.tile([128, FD], f32, tag=f"s1_{f % 2}", name=f"s1_{f}")
                    nc.scalar.square(s1[:], t1f[:])
                    tp1f = scr.tile([128, FD], f32, tag=f"tp1f{f % 2}", name=f"tp1f_{f}")
                    nc.vector.tensor_scalar(tp1f[:], s1[:], -1.0, 1.0, ALU.mult, ALU.add)
                    tp1h = persist.tile([128, FD], f16, tag=f"tp1h_{f}", name=f"tp1h_{f}")
                    nc.scalar.copy(tp1h[:], tp1f[:])
                    tp1l = persist.tile([128, FD], f16, tag=f"tp1l_{f}", name=f"tp1l_{f}")
                    TT(nc.vector, tp1l[:], tp1f[:], tp1h[:], ALU.subtract)
                    g2m = persist.tile([128, FD], f16, tag=f"g2m_{f}", name=f"g2m_{f}")
                    TT(nc.vector, g2m[:], t1f[:], tp1f[:])
                    g3m = persist.tile([128, FD], f16, tag=f"g3m_{f}", name=f"g3m_{f}")
                    nc.vector.scalar_tensor_tensor(
                        g3m[:], s1[:], 1.0 / 3.0, tp1f[:], ALU.subtract, ALU.mult)
                    l1[f] = (t1h, t1l, tp1h, tp1l, g2m, g3m)

            # ---- local reduce + AllReduce (gpsimd queue is empty -> fast trigger)
            b8sb = smp.tile([8, FD], f32, name="b8sb")
            nc.scalar.copy(b8sb[:], b8[:])
            bcol = ps.tile([128, 1], f32, tag="pBC", name="bcol")
            for j in range(4):
                nc.tensor.matmul(bcol[:], b8sb[:, j * 128:(j + 1) * 128],
                                 smt["sel4m"][:, j:j + 1],
                                 start=(j == 0), stop=(j == 3))
            b_loc = smp.tile([128, 1], f32, name="bloc")
            nc.scalar.copy(b_loc[:], bcol[:])
            nc.sync.dma_start(cc_in.ap(), b_loc[:])
            nc.gpsimd.collective_compute(
                "AllReduce", ALU.add,
                replica_groups=[list(range(NCORES))],
                ins=[cc_in.ap()], outs=[cc_out.ap()],
            )
            b_ar = smp.tile([128, 1], f32, name="bar")
            nc.sync.dma_start(b_ar[:], cc_out.ap())

            # ---- trunk layer-2 wave (fills AllReduce latency) ----
            pers = {}
            for f in range(NTRUNK):
                t1h, t1l, tp1h, tp1l, g2m, g3m = l1[f]
                z2 = ps.tile([128, FD], f32, tag="pT0", name=f"z2_{f}")
                nc.tensor.matmul(z2[:], smt["wt2h"][:], t1h[:], start=True, stop=False)
                nc.tensor.matmul(z2[:], smt["wt2h"][:], t1l[:], start=False, stop=False)
                nc.tensor.matmul(z2[:], smt["wt2l"][:], t1h[:], start=False, stop=True)
                A = ps.tile([128, FD], f32, tag="pT1", name=f"A_{f}")
                nc.tensor.matmul(A[:], smt["w2ah"][:], tp1h[:], start=True, stop=False)
                nc.tensor.matmul(A[:], smt["w2ah"][:], tp1l[:], start=False, stop=False)
                nc.tensor.matmul(A[:], smt["w2al"][:], tp1h[:], start=False, stop=True)
                B = ps.tile([128, FD], f32, tag="pB", name=f"B_{f}")
                nc.tensor.matmul(B[:], smt["w2bh"][:], g2m[:], start=True, stop=False)
                nc.tensor.matmul(B[:], smt["w2bl"][:], g2m[:], start=False, stop=True)
                C = ps.tile([128, FD], f32, tag="pC", name=f"C_{f}")
                nc.tensor.matmul(C[:], smt["w2ch"][:], g3m[:], start=True, stop=False)
                nc.tensor.matmul(C[:], smt["w2cl"][:], g3m[:], start=False, stop=True)

                t2f = scr.tile([128, FD], f32, tag="t2f", name=f"t2f_{f}")
                nc.scalar.activation(t2f[:], z2[:], AF.Tanh, bias=smt["bt2b"][:])
                t2h = persist.tile([128, FD], f16, tag=f"t1h_{f}", name=f"t2h_{f}")
                nc.scalar.copy(t2h[:], t2f[:])
                t2l = persist.tile([128, FD], f16, tag=f"t1l_{f}", name=f"t2l_{f}")
                TT(nc.vector, t2l[:], t2f[:], t2h[:], ALU.subtract)
                s2 = scr.tile([128, FD], f32, tag="s2", name=f"s2_{f}")
                nc.scalar.square(s2[:], t2f[:])
                tp2 = scr.tile([128, FD], f32, tag="tp2", name=f"tp2_{f}")
                nc.vector.tensor_scalar(tp2[:], s2[:], -1.0, 1.0, ALU.mult, ALU.add)
                A2 = scr.tile([128, FD], f32, tag="A2", name=f"A2_{f}")
                nc.scalar.square(A2[:], A[:])
                P1f = scr.tile([128, FD], f32, tag="P1f", name=f"P1f_{f}")
                TT(nc.vector, P1f[:], tp2[:], A[:])
                P1h = persist.tile([128, FD], f16, tag=f"tp1h_{f}", name=f"P1h_{f}")
                nc.scalar.copy(P1h[:], P1f[:])
                P1l = persist.tile([128, FD], f16, tag=f"tp1l_{f}", name=f"P1l_{f}")
                TT(nc.vector, P1l[:], P1f[:], P1h[:], ALU.subtract)
                M4 = scr.tile([128, FD], f32, tag="M4", name=f"M4_{f}")
                TT(nc.gpsimd, M4[:], tp2[:], A2[:])
                M5 = scr.tile([128, FD], f32, tag="M5", name=f"M5_{f}")
                TT(nc.gpsimd, M5[:], t2f[:], M4[:])
                M6 = scr.tile([128, FD], f32, tag="M6", name=f"M6_{f}")
                TT(nc.vector, M6[:], tp2[:], B[:])
                uxxMf = scr.tile([128, FD], f32, tag="ux2f", name=f"ux2f_{f}")
                nc.vector.scalar_tensor_tensor(
                    uxxMf[:], M5[:], -2.0, M6[:], ALU.mult, ALU.add)
                ux2h = persist.tile([128, FD], f16, tag=f"g2m_{f}", name=f"ux2h_{f}")
                nc.scalar.copy(ux2h[:], uxxMf[:])
                ux2l = persist.tile([128, FD], f16, tag=f"g3m_{f}", name=f"ux2l_{f}")
                TT(nc.vector, ux2l[:], uxxMf[:], ux2h[:], ALU.subtract)
                A3 = scr.tile([128, FD], f32, tag="A3", name=f"A3_{f}")
                TT(nc.vector, A3[:], A2[:], A[:])
                V = scr.tile([128, FD], f32, tag="V", name=f"V_{f}")
                nc.vector.scalar_tensor_tensor(
                    V[:], s2[:], 1.0 / 3.0, tp2[:], ALU.subtract, ALU.mult)
                M1 = scr.tile([128, FD], f32, tag="M1", name=f"M1_{f}")
                TT(nc.gpsimd, M1[:], V[:], A3[:])
                W1 = scr.tile([128, FD], f32, tag="W1", name=f"W1_{f}")
                TT(nc.vector, W1[:], P1f[:], B[:])
                M2 = scr.tile([128, FD], f32, tag="M2", name=f"M2_{f}")
                TT(nc.gpsimd, M2[:], t2f[:], W1[:])
                M3 = scr.tile([128, FD], f32, tag="M3", name=f"M3_{f}")
                TT(nc.vector, M3[:], tp2[:], C[:])
                D1 = scr.tile([128, FD], f32, tag="D1", name=f"D1_{f}")
                TT(nc.gpsimd, D1[:], M1[:], M2[:], ALU.subtract)
                uxxxMf = scr.tile([128, FD], f32, tag="ux3f", name=f"ux3f_{f}")
                nc.vector.scalar_tensor_tensor(
                    uxxxMf[:], D1[:], 6.0, M3[:], ALU.mult, ALU.add)
                ux3h = persist.tile([128, FD], f16, tag=f"ux3h_{f}", name=f"ux3h_{f}")
                nc.scalar.copy(ux3h[:], uxxxMf[:])
                ux3l = persist.tile([128, FD], f16, tag=f"ux3l_{f}", name=f"ux3l_{f}")
                TT(nc.vector, ux3l[:], uxxxMf[:], ux3h[:], ALU.subtract)
                pers[f] = (t2h, t2l, P1h, P1l, ux2h, ux2l, ux3h, ux3l)

            # ---- c = Wt3^T b as fp16 hi/lo pair stationary ----
            bpair = smp.tile([128, 2], f16, name="bpair")
            nc.scalar.copy(bpair[:, 0:1], b_ar[:])
            TT(nc.vector, bpair[:, 1:2], b_ar[:], bpair[:, 0:1], ALU.subtract)
            c0p = ps.tile([1, 128], f32, tag="pT0", name="c0p")
            nc.tensor.matmul(c0p[:], bpair[:, 0:1], smt["wt3h"][:], start=True, stop=False)
            nc.tensor.matmul(c0p[:], bpair[:, 1:2], smt["wt3h"][:], start=False, stop=False)
            nc.tensor.matmul(c0p[:], bpair[:, 0:1], smt["wt3l"][:], start=False, stop=True)
            c0 = smp.tile([1, 128], f32, name="c0")
            nc.scalar.copy(c0[:], c0p[:])
            ct = ps.tile([128, 1], f32, tag="pT1", name="ct")
            nc.tensor.matmul(ct[:], c0[:], ones11[:], start=True, stop=True)
            cpair = smp.tile([128, 2], f16, name="cpair")
            nc.scalar.copy(cpair[:, 0:1], ct[:])
            TT(nc.vector, cpair[:, 1:2], ct[:], cpair[:, 0:1], ALU.subtract)

            # ---- energy phase ----
            for e in range(NEN):
                pA_ = pers[e]
                pB_ = pers[e + NEN]
                # extract u, ux, uxx, uxxx for both halves; build hi/lo row stacks
                mov_e = scr.tile([8, FD], f16, tag="mv_e", name=f"mve_{e}")
                mov_p = scr.tile([8, FD], f16, tag="mv_p", name=f"mvp_{e}")
                mov_pp = scr.tile([8, FD], f16, tag="mv_pp", name=f"mvpp_{e}")
                for hx, (th, tl, Ph, Pl, x2h, x2l, x3h, x3l) in ((0, pA_), (1, pB_)):
                    hls = []
                    for qi, (Xh, Xl) in enumerate(
                            ((th, tl), (Ph, Pl), (x2h, x2l), (x3h, x3l))):
                        uq = ps.tile([1, FD], f32, tag=["pD", "pE"][qi % 2],
                                     name=f"uq{e}_{hx}_{qi}")
                        nc.tensor.matmul(uq[:], cpair[:, 0:1], Xh[:], start=True, stop=False)
                        nc.tensor.matmul(uq[:], cpair[:, 1:2], Xh[:], start=False, stop=False)
                        nc.tensor.matmul(uq[:], cpair[:, 0:1], Xl[:], start=False, stop=True)
                        hl = scr.tile([1, 2 * FD], f16, tag=f"hl{qi}_{hx}",
                                      name=f"hl{e}_{hx}_{qi}")
                        nc.scalar.copy(hl[:, 0:FD], uq[:])
                        TT(nc.vector, hl[:, FD:2 * FD], uq[:], hl[:, 0:FD], ALU.subtract)
                        hls.append(hl)
                    r0 = 4 * hx
                    nc.sync.dma_start(mov_e[r0:r0 + 2, :], hls[0][:])
                    nc.sync.dma_start(mov_e[r0 + 2:r0 + 4, :], hls[1][:])
                    nc.sync.dma_start(mov_p[r0:r0 + 2, :], hls[1][:])
                    nc.sync.dma_start(mov_p[r0 + 2:r0 + 4, :], hls[2][:])
                    nc.sync.dma_start(mov_pp[r0:r0 + 2, :], hls[2][:])
                    nc.sync.dma_start(mov_pp[r0 + 2:r0 + 4, :], hls[3][:])

                z1e = ps.tile([128, FD], f32, tag="pB", name=f"z1e_{e}")
                nc.tensor.matmul(z1e[:], smt["SH"][:], mov_e[:], start=True, stop=False)
                nc.tensor.matmul(z1e[:], smt["SL"][:], mov_e[:], start=False, stop=True)
                z1p = ps.tile([128, FD], f32, tag="pC", name=f"z1p_{e}")
                nc.tensor.matmul(z1p[:], smt["SH"][:], mov_p[:], start=True, stop=False)
                nc.tensor.matmul(z1p[:], smt["SL"][:], mov_p[:], start=False, stop=True)
                z1pp = ps.tile([128, FD], f32, tag="pBC", name=f"z1pp_{e}")
                nc.tensor.matmul(z1pp[:], smt["SH"][:], mov_pp[:], start=True, stop=False)
                nc.tensor.matmul(z1pp[:], smt["SL"][:], mov_pp[:], start=False, stop=True)

                t1ef = scr.tile([128, FD], f32, tag="t2f", name=f"t1ef_{e}")
                nc.scalar.activation(t1ef[:], z1e[:], AF.Tanh, bias=smt["be1b2"][:])
                t1eh = scr.tile([128, FD], f16, tag="t1eh", name=f"t1eh_{e}")
                nc.scalar.copy(t1eh[:], t1ef[:])
                t1el = scr.tile([128, FD], f16, tag="t1el", name=f"t1el_{e}")
                TT(nc.vector, t1el[:], t1ef[:], t1eh[:], ALU.subtract)
                s1e = scr.tile([128, FD], f32, tag="s2", name=f"s1e_{e}")
                nc.scalar.square(s1e[:], t1ef[:])
                m_ = scr.tile([128, FD], f16, tag="m_", name=f"m_{e}")
                nc.vector.tensor_scalar(m_[:], s1e[:], -1.0, 1.0, ALU.mult, ALU.add)
                z1p2 = scr.tile([128, FD], f32, tag="tp2", name=f"z1p2_{e}")
                nc.scalar.square(z1p2[:], z1p[:])
                N1 = scr.tile([128, FD], f32, tag="A2", name=f"N1_{e}")
                TT(nc.gpsimd, N1[:], t1ef[:], m_[:])
                a1p = scr.tile([128, FD], f16, tag="a1p", name=f"a1p_{e}")
                TT(nc.vector, a1p[:], m_[:], z1p[:])
                N2 = scr.tile([128, FD], f32, tag="P1f", name=f"N2_{e}")
                TT(nc.gpsimd, N2[:], N1[:], z1p2[:])
                N3 = scr.tile([128, FD], f32, tag="M4", name=f"N3_{e}")
                TT(nc.vector, N3[:], m_[:], z1pp[:])
                zin = scr.tile([128, FD], f16, tag="zin", name=f"zin_{e}")
                nc.vector.scalar_tensor_tensor(
                    zin[:], N2[:], -2.0, N3[:], ALU.mult, ALU.add)
                mpc = scr.tile([128, FD], f16, tag="mpc", name=f"mpc_{e}")
                TT(nc.vector, mpc[:], N1[:], z1p[:])
                O1 = scr.tile([128, FD], f32, tag="M5", name=f"O1_{e}")
                nc.vector.scalar_tensor_tensor(
                    O1[:], s1e[:], 1.0 / 3.0, m_[:], ALU.subtract, ALU.mult)
                O2f = scr.tile([128, FD], f32, tag="M6", name=f"O2f_{e}")
                TT(nc.gpsimd, O2f[:], O1[:], z1p2[:])
                O3f = scr.tile([128, FD], f32, tag="ux2f", name=f"O3f_{e}")
                TT(nc.vector, O3f[:], N1[:], z1pp[:])
                O2m = scr.tile([128, FD], f16, tag="O2m", name=f"O2m_{e}")
                nc.vector.scalar_tensor_tensor(
                    O2m[:], O2f[:], 3.0, O3f[:], ALU.mult, ALU.subtract)

                z2e = ps.tile([128, FD], f32, tag="pD", name=f"z2e_{e}")
                nc.tensor.matmul(z2e[:], smt["e0"][:], t1eh[:], start=True, stop=False)
                nc.tensor.matmul(z2e[:], smt["e0"][:], t1el[:], start=False, stop=True)
                z2ep = ps.tile([128, FD], f32, tag="pE", name=f"z2ep_{e}")
                nc.tensor.matmul(z2ep[:], smt["e0"][:], a1p[:], start=True, stop=True)
                z2epp = ps.tile([128, FD], f32, tag="pT0", name=f"z2epp_{e}")
                nc.tensor.matmul(z2epp[:], smt["e0"][:], zin[:], start=True, stop=True)
                Dz = ps.tile([128, FD], f32, tag="pT1", name=f"Dz_{e}")
                nc.tensor.matmul(Dz[:], smt["eq"][:], m_[:], start=True, stop=True)
                DyN = ps.tile([128, FD], f32, tag="pB", name=f"DyN_{e}")
                nc.tensor.matmul(DyN[:], smt["ep"][:], m_[:], start=True, stop=True)
                DzpN = ps.tile([128, FD], f32, tag="pC", name=f"DzpN_{e}")
                nc.tensor.matmul(DzpN[:], smt["eq"][:], mpc[:], start=True, stop=True)
                DypN = ps.tile([128, FD], f32, tag="pBC", name=f"DypN_{e}")
                nc.tensor.matmul(DypN[:], smt["ep"][:], mpc[:], start=True, stop=True)
                Dzpp2 = ps.tile([128, FD], f32, tag="pMV", name=f"Dzpp2_{e}")
                nc.tensor.matmul(Dzpp2[:], smt["eq"][:], O2m[:], start=True, stop=True)

                t2e = scr.tile([128, FD], f32, tag="A3", name=f"t2e_{e}")
                nc.scalar.activation(t2e[:], z2e[:], AF.Tanh, bias=smt["be2b2"][:])
                s2e = scr.tile([128, FD], f32, tag="V", name=f"s2e_{e}")
                nc.scalar.square(s2e[:], t2e[:])
                w_ = scr.tile([128, FD], f32, tag="M1", name=f"w_{e}")
                nc.vector.tensor_scalar(w_[:], s2e[:], -1.0, 1.0, ALU.mult, ALU.add)
                z2ep2 = scr.tile([128, FD], f32, tag="W1", name=f"z2ep2_{e}")
                nc.scalar.square(z2ep2[:], z2ep[:])
                Q1 = scr.tile([128, FD], f32, tag="M2", name=f"Q1_{e}")
                TT(nc.gpsimd, Q1[:], t2e[:], w_[:])
                wpc = scr.tile([128, FD], f16, tag="wpc", name=f"wpc_{e}")
                TT(nc.vector, wpc[:], Q1[:], z2ep[:])
                R1 = scr.tile([128, FD], f32, tag="M3", name=f"R1_{e}")
                nc.vector.scalar_tensor_tensor(
                    R1[:], s2e[:], 1.0 / 3.0, w_[:], ALU.subtract, ALU.mult)
                R2f = scr.tile([128, FD], f32, tag="D1", name=f"R2f_{e}")
                TT(nc.gpsimd, R2f[:], R1[:], z2ep2[:])
                R3f = scr.tile([128, FD], f32, tag="ux3f", name=f"R3f_{e}")
                TT(nc.vector, R3f[:], Q1[:], z2epp[:])
                t1m = scr.tile([128, FD], f32, tag="s1_0", name=f"t1m_{e}")
                nc.vector.scalar_tensor_tensor(
                    t1m[:], R2f[:], 3.0, R3f[:], ALU.mult, ALU.subtract)
                F1 = scr.tile([128, FD], f16, tag="F1", name=f"F1_{e}")
                TT(nc.vector, F1[:], t1m[:], Dz[:])
                DyNs = scr.tile([128, FD], f32, tag="t1f0", name=f"DyNs_{e}")
                nc.scalar.copy(DyNs[:], DyN[:])
                t2m = scr.tile([128, FD], f32, tag="s1_1", name=f"t2m_{e}")
                nc.vector.scalar_tensor_tensor(
                    t2m[:], DzpN[:], 4.0, DyNs[:], ALU.mult, ALU.add)
                F2 = scr.tile([128, FD], f16, tag="F2", name=f"F2_{e}")
                TT(nc.gpsimd, F2[:], wpc[:], t2m[:])
                DypNs = scr.tile([128, FD], f32, tag="t1f1", name=f"DypNs_{e}")
                nc.scalar.copy(DypNs[:], DypN[:])
                t3m = scr.tile([128, FD], f32, tag="tp1f0", name=f"t3m_{e}")
                TT(nc.vector, t3m[:], Dzpp2[:], DypNs[:], ALU.add)
                F3 = scr.tile([128, FD], f16, tag="F3", name=f"F3_{e}")
                TT(nc.vector, F3[:], w_[:], t3m[:])

                vps = ps.tile([2, FD], f32, tag="pBC", name=f"vps_{e}")
                nc.tensor.matmul(vps[:], smt["v6"][:, 0:2], F1[:], start=True, stop=False)
                nc.tensor.matmul(vps[:], smt["v6"][:, 2:4], F2[:], start=False, stop=False)
                nc.tensor.matmul(vps[:], smt["v6"][:, 4:6], F3[:], start=False, stop=True)
                ot = scr.tile([2, FD], f32, tag="ot", name=f"ot_{e}")
                nc.scalar.copy(ot[:], vps[:])
                nc.sync.dma_start(out_d.ap()[:, e * FD:(e + 1) * FD], ot[:])

    nc.compile()
    return nc


def _get_nc():
    if "nc" not in _CACHE:
        _CACHE["nc"] = _build()
    return _CACHE["nc"]


def kernel(**inputs):
    import concourse.bass_utils as bass_utils

    f = lambda k: np.asarray(inputs[k], np.float32)
    a, x, t = f("a"), f("x"), np.float32(inputs["t"])
    Wb, Wt1, bt1, Wt2, bt2 = f("Wb"), f("Wt1"), f("bt1"), f("Wt2"), f("bt2")
    Wt3, We1, be1, We2, be2, We3 = (
        f("Wt3"), f("We1"), f("be1"), f("We2"), f("be2"), f("We3"))

    h16 = lambda v: np.asarray(v, np.float32).astype(np.float16)
    def pair16(v):
        h = h16(v)
        return h, h16(np.asarray(v, np.float32) - h.astype(np.float32))

    w1 = Wt1[:, 0]
    c1b = (Wt1[:, 1] * t + bt1)[:, None]
    w1h, w1l = pair16(w1)
    w11 = np.stack([w1h, w1h, w1l, w1l])                       # [4,128]
    wt2t = np.ascontiguousarray(Wt2.T)
    mk = lambda M: pair16(M)
    wt2h, wt2l = mk(wt2t)
    w2ah, w2al = mk(wt2t * w1[:, None])
    w2bh, w2bl = mk(wt2t * (-2.0 * w1 ** 2)[:, None])
    w2ch, w2cl = mk(wt2t * (6.0 * w1 ** 3)[:, None])
    wt3h, wt3l = mk(Wt3)

    p, q, v = We1[:, 0], We1[:, 1], We3[0]
    ph, pl = pair16(p)
    qh, ql = pair16(q)
    SH = np.zeros((8, 128), np.float16)
    SL = np.zeros((8, 128), np.float16)
    for blk_i, cs in ((0, slice(0, 64)), (4, slice(64, 128))):
        SH[blk_i + 0, cs] = ph; SH[blk_i + 1, cs] = ph
        SH[blk_i + 2, cs] = qh; SH[blk_i + 3, cs] = qh
        SL[blk_i + 0, cs] = pl; SL[blk_i + 2, cs] = ql
    blk = lambda M: np.block([[M, np.zeros_like(M)], [np.zeros_like(M), M]])
    We2T = We2.T
    e0 = h16(blk(We2T))
    eq = h16(blk(We2T * q[:, None]))
    ep = h16(blk(We2T * p[:, None]))
    v6 = np.zeros((128, 6), np.float16)
    for i in range(3):
        v6[0:64, 2 * i] = h16(2.0 * v)
        v6[64:128, 2 * i + 1] = h16(2.0 * v)
    sel4m = np.zeros((8, 4), np.float32)
    for j in range(4):
        sel4m[2 * j, j] = 1.0
        sel4m[2 * j + 1, j] = 1.0

    smalls = {
        "w11": w11, "c1b": c1b.astype(np.float32), "bt2b": bt2[:, None].astype(np.float32),
        "wt2h": wt2h, "wt2l": wt2l, "w2ah": w2ah, "w2al": w2al,
        "w2bh": w2bh, "w2bl": w2bl, "w2ch": w2ch, "w2cl": w2cl,
        "wt3h": wt3h, "wt3l": wt3l, "SH": SH, "SL": SL,
        "e0": e0, "eq": eq, "ep": ep, "v6": v6,
        "be1b2": np.concatenate([be1, be1])[:, None].astype(np.float32),
        "be2b2": np.concatenate([be2, be2])[:, None].astype(np.float32),
        "sel4m": sel4m,
    }
    smalls = {k: np.ascontiguousarray(val) for k, val in smalls.items()}

    in_maps = []
    for c in range(NCORES):
        blk_w = Wb[:, c * KSH:(c + 1) * KSH]                   # [128, 65536]
        tr = blk_w.T.reshape(NKT, 128, 128).transpose(1, 0, 2)  # [k1, kt, p]
        tr = tr.reshape(128, NCHUNK, KTC * 128).transpose(1, 0, 2)
        wsh = np.ascontiguousarray(h16(1024.0 * tr))           # [32,128,2048]
        ash = (a[c * KSH:(c + 1) * KSH] / 1024.0).reshape(NKT, 128).T  # [k1, kt]
        ah, al = pair16(ash)
        a2 = np.ascontiguousarray(np.stack([ah, al], axis=2))  # [128,512,2]
        xs = x[c * NPTS:(c + 1) * NPTS]
        xh, xl = pair16(xs)
        x4 = np.ascontiguousarray(np.stack([xh, xl, xh, xl]))  # [4,4096]
        im = {"w": wsh, "a2": a2, "x4": x4}
        im.update(smalls)
        in_maps.append(im)

    global _last_in_maps
    _last_in_maps = in_maps
    nc = _get_nc()
    res = bass_utils.run_bass_kernel_spmd(nc, in_maps, core_ids=list(range(NCORES)))
    outs = []
    for c in range(NCORES):
        o = res.results[c]["out"]          # [2, NPTS//2]
        outs.append(np.asarray(o).reshape(-1))
    return np.concatenate(outs).astype(np.float32)


# revision 22
# speedup vs baseline: 1.3698x; 1.0089x over previous
"""Bass/Trainium2 kernel for nn_HNO_37065567764989 (self-contained).

Strategy (8 NeuronCores, SPMD):
- Branch matvec b = Wb@a column-sharded 8 ways. Each core streams its 16MB
  shard as fp16 (W scaled by 2^10 to stay normal; a carried as an fp16 hi/lo
  stationary pair, M=8 batched over 4 k-tiles per matmul). 512B AllReduce
  combines partials.
- Nx=32768 points sharded 8 ways (4096/core). Trunk layer-1 overlaps the Wb
  stream; layer-2 (z2/A/B/C, fp16 hi/lo pair stationaries+movings) fills the
  AllReduce latency; EnergyNet runs after, extracting u/u_x/u_xx/u_xxx rows
  with a c=Wt3^T b fp16 stationary and building first-layer preactivations
  from stacked hi/lo row movings (2 matmuls each via host (p,q) stationaries).
- High-sensitivity values flow as fp16 hi/lo pairs (~2^-24); low-sensitivity
  operands are single fp16. All matmuls run at 1 cy/row.
"""
import sys

for _p in ("/opt/trn_rl_repo",):
    if _p not in sys.path:
        sys.path.insert(0, _p)

import numpy as np

MP1, NX, P, HT, HE = 524288, 32768, 128, 128, 64
NCORES = 8
KSH = MP1 // NCORES        # 65536 contraction elems per core
NKT = KSH // 128           # 512 k-tiles
NCHUNK = 16
KTC = NKT // NCHUNK        # 32 k-tiles per chunk
NPTS = NX // NCORES        # 4096 points per core
FD = 512
NTRUNK = NPTS // FD        # 8 trunk tiles
NEN = NTRUNK // 2          # 4 energy tiles (two halves stacked)

_CACHE = {}


def _build():
    import concourse.bacc as bacc
    import concourse.mybir as mybir
    from concourse import tile

    f32 = mybir.dt.float32
    f16 = mybir.dt.float16
    AF = mybir.ActivationFunctionType
    ALU = mybir.AluOpType

    nc = bacc.Bacc("TRN2", target_bir_lowering=False, debug=False,
                   num_devices=NCORES)

    w_d = nc.dram_tensor("w", [NCHUNK, 128, KTC * 128], f16, kind="ExternalInput")
    a_d = nc.dram_tensor("a2", [128, NKT, 2], f16, kind="ExternalInput")
    x_d = nc.dram_tensor("x4", [4, NPTS], f16, kind="ExternalInput")
    sm = {}
    for name, shape, dt in [
        ("w11", [4, 128], f16), ("c1b", [128, 1], f32), ("bt2b", [128, 1], f32),
        ("wt2h", [128, 128], f16), ("wt2l", [128, 128], f16),
        ("w2ah", [128, 128], f16), ("w2al", [128, 128], f16),
        ("w2bh", [128, 128], f16), ("w2bl", [128, 128], f16),
        ("w2ch", [128, 128], f16), ("w2cl", [128, 128], f16),
        ("wt3h", [128, 128], f16), ("wt3l", [128, 128], f16),
        ("SEH", [12, 128], f16), ("SEL", [12, 128], f16),
        ("SPH", [12, 128], f16), ("SPL", [12, 128], f16),
        ("SPPH", [12, 128], f16), ("SPPL", [12, 128], f16),
        ("e0", [128, 128], f16), ("eq", [128, 128], f16), ("ep", [128, 128], f16),
        ("v6", [128, 6], f16),
        ("be1b2", [128, 1], f32), ("be2b2", [128, 1], f32),
        ("sel4m", [8, 4], f32),
    ]:
        sm[name] = nc.dram_tensor(name, shape, dt, kind="ExternalInput")
    out_d = nc.dram_tensor("out", [2, NPTS // 2], f32, kind="ExternalOutput")
    cc_in = nc.dram_tensor("cc_in", [128, 1], f32)
    cc_out = nc.dram_tensor("cc_out", [128, 1], f32, addr_space="Shared")

    def TT(eng, out, i0, i1, op=ALU.mult):
        eng.tensor_tensor(out, i0, i1, op)

    with tile.TileContext(nc) as tc:
        with (
            tc.tile_pool(name="smp", bufs=1) as smp,
            tc.tile_pool(name="persist", bufs=1) as persist,
            tc.tile_pool(name="wpool", bufs=2) as wpool,
            tc.tile_pool(name="scr", bufs=1) as scr,
            tc.tile_pool(name="ps8", bufs=1, space="PSUM") as ps,
        ):
            smt = {}
            for name, h in sm.items():
                t = smp.tile(list(h.shape), h.dtype, name=f"sb_{name}")
                nc.sync.dma_start(t[:], h.ap())
                smt[name] = t
            x4 = smp.tile([4, NPTS], f16, name="x4t")
            nc.sync.dma_start(x4[:], x_d.ap())
            a2 = smp.tile([128, NKT, 2], f16, name="a2t")
            nc.sync.dma_start(a2[:], a_d.ap())
            ones11 = smp.tile([1, 1], f32, name="ones11")
            nc.vector.memset(ones11[:], 1.0)

            # ---- trunk layer-1 z1 matmuls ----
            z1tags = ["pT0", "pT1", "pB", "pC"]
            z1ps = []
            for f in range(NTRUNK):
                cs = slice(f * FD, (f + 1) * FD)
                z1 = ps.tile([128, FD], f32, tag=z1tags[f % 4], name=f"z1_{f}")
                nc.tensor.matmul(z1[:], smt["w11"][:], x4[:, cs], start=True, stop=True)
                z1ps.append(z1)

            # ---- matvec: stream W shard; layer-1 elementwise rides along ----
            l1 = {}
            b8 = ps.tile([8, FD], f32, tag="pMV", name="b8")
            for i in range(NCHUNK):
                wch = wpool.tile([128, KTC * 128], f16, tag="wch", name="wch")
                nc.sync.dma_start(wch[:], w_d.ap()[i])
                for g in range(KTC // 4):
                    nc.tensor.matmul(
                        b8[:], a2[:, i * KTC + 4 * g:i * KTC + 4 * (g + 1), :],
                        wch[:, g * 512:(g + 1) * 512],
                        start=(i == 0 and g == 0),
                        stop=(i == NCHUNK - 1 and g == KTC // 4 - 1),
                    )
                if i < NTRUNK:
                    f = i
                    z1 = z1ps[f]
                    t1f = scr.tile([128, FD], f32, tag=f"t1f{f % 2}", name=f"t1f_{f}")
                    nc.scalar.activation(t1f[:], z1[:], AF.Tanh, bias=smt["c1b"][:])
                    t1h = persist.tile([128, FD], f16, tag=f"t1h_{f}", name=f"t1h_{f}")
                    nc.scalar.copy(t1h[:], t1f[:])
                    t1l = persist.tile([128, FD], f16, tag=f"t1l_{f}", name=f"t1l_{f}")
                    TT(nc.vector, t1l[:], t1f[:], t1h[:], ALU.subtract)
                    s1 = scr.tile([128, FD], f32, tag=f"s1_{f % 2}", name=f"s1_{f}")
                    nc.scalar.square(s1[:], t1f[:])
                    tp1f = scr.tile([128, FD], f32, tag=f"tp1f{f % 2}", name=f"tp1f_{f}")
                    nc.vector.tensor_scalar(tp1f[:], s1[:], -1.0, 1.0, ALU.mult, ALU.add)
                    tp1h = persist.tile([128, FD], f16, tag=f"tp1h_{f}", name=f"tp1h_{f}")
                    nc.scalar.copy(tp1h[:], tp1f[:])
                    tp1l = persist.tile([128, FD], f16, tag=f"tp1l_{f}", name=f"tp1l_{f}")
                    TT(nc.vector, tp1l[:], tp1f[:], tp1h[:], ALU.subtract)
                    g2m = persist.tile([128, FD], f16, tag=f"g2m_{f}", name=f"g2m_{f}")
                    TT(nc.vector, g2m[:], t1f[:], tp1f[:])
                    g3m = persist.tile([128, FD], f16, tag=f"g3m_{f}", name=f"g3m_{f}")
                    nc.vector.scalar_tensor_tensor(
                        g3m[:], s1[:], 1.0 / 3.0, tp1f[:], ALU.subtract, ALU.mult)
                    l1[f] = (t1h, t1l, tp1h, tp1l, g2m, g3m)

            # ---- local reduce + AllReduce ----
            b8sb = smp.tile([8, FD], f32, name="b8sb")
            nc.scalar.copy(b8sb[:], b8[:])
            bcol = ps.tile([128, 1], f32, tag="pBC", name="bcol")
            for j in range(4):
                nc.tensor.matmul(bcol[:], b8sb[:, j * 128:(j + 1) * 128],
                                 smt["sel4m"][:, j:j + 1],
                                 start=(j == 0), stop=(j == 3))
            b_loc = smp.tile([128, 1], f32, name="bloc")
            nc.scalar.copy(b_loc[:], bcol[:])
            nc.sync.dma_start(cc_in.ap(), b_loc[:])
            nc.gpsimd.collective_compute(
                "AllReduce", ALU.add,
                replica_groups=[list(range(NCORES))],
                ins=[cc_in.ap()], outs=[cc_out.ap()],
            )
            b_ar = smp.tile([128, 1], f32, name="bar")
            nc.sync.dma_start(b_ar[:], cc_out.ap())

            # ---- trunk layer-2 wave (fills AllReduce latency) ----
            # stage-2 outputs for trunk tiles f and f+4 share one [128,1024]
            # tile (halves side by side in the free dim) so energy extracts
            # can read both halves of an energy tile from one tile family.
            sh = {}
            for f in range(NTRUNK):
                j, off = f % NEN, (f // NEN) * FD
                t1h, t1l, tp1h, tp1l, g2m, g3m = l1[f]
                if f < NEN:
                    sh[j] = tuple(
                        persist.tile([128, 2 * FD], f16, tag=f"sh{nm}_{j}",
                                     name=f"sh_{nm}_{j}")
                        for nm in ("t2h", "t2l", "P1h", "P1l", "ux2", "ux3"))
                t2h_s, t2l_s, P1h_s, P1l_s, ux2_s, ux3_s = sh[j]
                osl = slice(off, off + FD)
                z2 = ps.tile([128, FD], f32, tag="pT0", name=f"z2_{f}")
                nc.tensor.matmul(z2[:], smt["wt2h"][:], t1h[:], start=True, stop=False)
                nc.tensor.matmul(z2[:], smt["wt2h"][:], t1l[:], start=False, stop=False)
                nc.tensor.matmul(z2[:], smt["wt2l"][:], t1h[:], start=False, stop=True)
                A = ps.tile([128, FD], f32, tag="pT1", name=f"A_{f}")
                nc.tensor.matmul(A[:], smt["w2ah"][:], tp1h[:], start=True, stop=False)
                nc.tensor.matmul(A[:], smt["w2ah"][:], tp1l[:], start=False, stop=False)
                nc.tensor.matmul(A[:], smt["w2al"][:], tp1h[:], start=False, stop=True)
                B = ps.tile([128, FD], f32, tag="pB", name=f"B_{f}")
                nc.tensor.matmul(B[:], smt["w2bh"][:], g2m[:], start=True, stop=False)
                nc.tensor.matmul(B[:], smt["w2bl"][:], g2m[:], start=False, stop=True)
                C = ps.tile([128, FD], f32, tag="pC", name=f"C_{f}")
                nc.tensor.matmul(C[:], smt["w2ch"][:], g3m[:], start=True, stop=False)
                nc.tensor.matmul(C[:], smt["w2cl"][:], g3m[:], start=False, stop=True)

                t2f = scr.tile([128, FD], f32, tag="t2f", name=f"t2f_{f}")
                nc.scalar.activation(t2f[:], z2[:], AF.Tanh, bias=smt["bt2b"][:])
                nc.scalar.copy(t2h_s[:, osl], t2f[:])
                TT(nc.vector, t2l_s[:, osl], t2f[:], t2h_s[:, osl], ALU.subtract)
                s2 = scr.tile([128, FD], f32, tag="s2", name=f"s2_{f}")
                nc.scalar.square(s2[:], t2f[:])
                tp2 = scr.tile([128, FD], f32, tag="tp2", name=f"tp2_{f}")
                nc.vector.tensor_scalar(tp2[:], s2[:], -1.0, 1.0, ALU.mult, ALU.add)
                A2 = scr.tile([128, FD], f32, tag="A2", name=f"A2_{f}")
                nc.scalar.square(A2[:], A[:])
                P1f = scr.tile([128, FD], f32, tag="P1f", name=f"P1f_{f}")
                TT(nc.vector, P1f[:], tp2[:], A[:])
                nc.scalar.copy(P1h_s[:, osl], P1f[:])
                TT(nc.vector, P1l_s[:, osl], P1f[:], P1h_s[:, osl], ALU.subtract)
                M4 = scr.tile([128, FD], f32, tag="M4", name=f"M4_{f}")
                TT(nc.gpsimd, M4[:], tp2[:], A2[:])
                M5 = scr.tile([128, FD], f32, tag="M5", name=f"M5_{f}")
                TT(nc.gpsimd, M5[:], t2f[:], M4[:])
                M6 = scr.tile([128, FD], f32, tag="M6", name=f"M6_{f}")
                TT(nc.vector, M6[:], tp2[:], B[:])
                nc.vector.scalar_tensor_tensor(
                    ux2_s[:, osl], M5[:], -2.0, M6[:], ALU.mult, ALU.add)
                A3 = scr.tile([128, FD], f32, tag="A3", name=f"A3_{f}")
                TT(nc.vector, A3[:], A2[:], A[:])
                V = scr.tile([128, FD], f32, tag="V", name=f"V_{f}")
                nc.vector.scalar_tensor_tensor(
                    V[:], s2[:], 1.0 / 3.0, tp2[:], ALU.subtract, ALU.mult)
                M1 = scr.tile([128, FD], f32, tag="M1", name=f"M1_{f}")
                TT(nc.gpsimd, M1[:], V[:], A3[:])
                W1 = scr.tile([128, FD], f32, tag="W1", name=f"W1_{f}")
                TT(nc.vector, W1[:], P1f[:], B[:])
                M2 = scr.tile([128, FD], f32, tag="M2", name=f"M2_{f}")
                TT(nc.gpsimd, M2[:], t2f[:], W1[:])
                M3 = scr.tile([128, FD], f32, tag="M3", name=f"M3_{f}")
                TT(nc.vector, M3[:], tp2[:], C[:])
                D1 = scr.tile([128, FD], f32, tag="D1", name=f"D1_{f}")
                TT(nc.gpsimd, D1[:], M1[:], M2[:], ALU.subtract)
                nc.vector.scalar_tensor_tensor(
                    ux3_s[:, osl], D1[:], 6.0, M3[:], ALU.mult, ALU.add)

            # ---- c = Wt3^T b (fp16 single stationary) ----
            b16 = smp.tile([128, 1], f16, name="b16")
            nc.scalar.copy(b16[:], b_ar[:])
            c0p = ps.tile([1, 128], f32, tag="pT0", name="c0p")
            nc.tensor.matmul(c0p[:], b16[:], smt["wt3h"][:], start=True, stop=False)
            nc.tensor.matmul(c0p[:], b16[:], smt["wt3l"][:], start=False, stop=True)
            c0 = smp.tile([1, 128], f32, name="c0")
            nc.scalar.copy(c0[:], c0p[:])
            ct = ps.tile([128, 1], f32, tag="pT1", name="ct")
            nc.tensor.matmul(ct[:], c0[:], ones11[:], start=True, stop=True)
            c16 = smp.tile([128, 1], f16, name="c16")
            nc.scalar.copy(c16[:], ct[:])

            # ---- energy phase ----
            exttags = ["pD", "pE", "pT0", "pT1"]
            for e in range(NEN):
                t2h_s, t2l_s, P1h_s, P1l_s, ux2_s, ux3_s = sh[e]
                mov12 = scr.tile([12, FD], f16, tag="mv12", name=f"mv12_{e}")
                ti = 0
                for qi, movs in enumerate(((t2h_s, t2l_s), (P1h_s, P1l_s),
                                           (ux2_s,), (ux3_s,))):
                    for hx in range(2):
                        osl = slice(hx * FD, (hx + 1) * FD)
                        uq = ps.tile([1, FD], f32, tag=exttags[ti % 4],
                                     name=f"uq{e}_{qi}_{hx}")
                        ti += 1
                        for mi, mv in enumerate(movs):
                            nc.tensor.matmul(uq[:], c16[:], mv[:, osl],
                                             start=(mi == 0),
                                             stop=(mi == len(movs) - 1))
                        if qi < 2:
                            # hi/lo pair packed side by side: [1, 2FD]
                            hl = scr.tile([1, 2 * FD], f16, tag=f"hl{qi}_{hx}",
                                          name=f"hl{e}_{qi}_{hx}")
                            nc.scalar.copy(hl[:, 0:FD], uq[:])
                            TT(nc.vector, hl[:, FD:2 * FD], uq[:], hl[:, 0:FD],
                               ALU.subtract)
                            nc.sync.dma_start(
                                mov12[qi * 4 + hx * 2:qi * 4 + hx * 2 + 2, :], hl[:])
                        else:
                            ex = scr.tile([1, FD], f16, tag=f"ex{qi}_{hx}",
                                          name=f"ex{e}_{qi}_{hx}")
                            nc.scalar.copy(ex[:], uq[:])
                            nc.sync.dma_start(
                                mov12[4 + qi * 2 + hx:5 + qi * 2 + hx, :], ex[:])

                z1e = ps.tile([128, FD], f32, tag="pB", name=f"z1e_{e}")
                nc.tensor.matmul(z1e[:], smt["SEH"][:], mov12[:], start=True, stop=False)
                nc.tensor.matmul(z1e[:], smt["SEL"][:], mov12[:], start=False, stop=True)
                z1p = ps.tile([128, FD], f32, tag="pC", name=f"z1p_{e}")
                nc.tensor.matmul(z1p[:], smt["SPH"][:], mov12[:], start=True, stop=False)
                nc.tensor.matmul(z1p[:], smt["SPL"][:], mov12[:], start=False, stop=True)
                z1pp = ps.tile([128, FD], f32, tag="pBC", name=f"z1pp_{e}")
                nc.tensor.matmul(z1pp[:], smt["SPPH"][:], mov12[:], start=True, stop=False)
                nc.tensor.matmul(z1pp[:], smt["SPPL"][:], mov12[:], start=False, stop=True)

                t1ef = scr.tile([128, FD], f32, tag="t2f", name=f"t1ef_{e}")
                nc.scalar.activation(t1ef[:], z1e[:], AF.Tanh, bias=smt["be1b2"][:])
                t1eh = scr.tile([128, FD], f16, tag="s2", name=f"t1eh_{e}")
                nc.scalar.copy(t1eh[:], t1ef[:])
                t1el = scr.tile([128, FD], f16, tag="tp2", name=f"t1el_{e}")
                TT(nc.vector, t1el[:], t1ef[:], t1eh[:], ALU.subtract)
                z1psb = scr.tile([128, FD], f16, tag="A2", name=f"z1psb_{e}")
                nc.scalar.copy(z1psb[:], z1p[:])
                z1ppsb = scr.tile([128, FD], f16, tag="P1f", name=f"z1ppsb_{e}")
                nc.scalar.copy(z1ppsb[:], z1pp[:])
                s1e = scr.tile([128, FD], f16, tag="M4", name=f"s1e_{e}")
                nc.scalar.square(s1e[:], t1ef[:])
                m_ = scr.tile([128, FD], f16, tag="M5", name=f"m_{e}")
                nc.vector.tensor_scalar(m_[:], s1e[:], -1.0, 1.0, ALU.mult, ALU.add)
                z1p2 = scr.tile([128, FD], f16, tag="M6", name=f"z1p2_{e}")
                TT(nc.gpsimd, z1p2[:], z1psb[:], z1psb[:])
                N1 = scr.tile([128, FD], f16, tag="A3", name=f"N1_{e}")
                TT(nc.gpsimd, N1[:], t1ef[:], m_[:])
                a1p = scr.tile([128, FD], f16, tag="V", name=f"a1p_{e}")
                TT(nc.vector, a1p[:], m_[:], z1psb[:])
                N2 = scr.tile([128, FD], f16, tag="M1", name=f"N2_{e}")
                TT(nc.gpsimd, N2[:], N1[:], z1p2[:])
                N3 = scr.tile([128, FD], f16, tag="W1", name=f"N3_{e}")
                TT(nc.vector, N3[:], m_[:], z1ppsb[:])
                zin = scr.tile([128, FD], f16, tag="M2", name=f"zin_{e}")
                nc.vector.scalar_tensor_tensor(
                    zin[:], N2[:], -2.0, N3[:], ALU.mult, ALU.add)
                mpc = scr.tile([128, FD], f16, tag="M3", name=f"mpc_{e}")
                TT(nc.vector, mpc[:], N1[:], z1psb[:])
                O1 = scr.tile([128, FD], f16, tag="D1", name=f"O1_{e}")
                nc.vector.scalar_tensor_tensor(
                    O1[:], s1e[:], 1.0 / 3.0, m_[:], ALU.subtract, ALU.mult)
                O2f = scr.tile([128, FD], f16, tag="t1f0", name=f"O2f_{e}")
                TT(nc.gpsimd, O2f[:], O1[:], z1p2[:])
                O3f = scr.tile([128, FD], f16, tag="t1f1", name=f"O3f_{e}")
                TT(nc.vector, O3f[:], N1[:], z1ppsb[:])
                O2m = scr.tile([128, FD], f16, tag="s1_0", name=f"O2m_{e}")
                nc.vector.scalar_tensor_tensor(
                    O2m[:], O2f[:], 3.0, O3f[:], ALU.mult, ALU.subtract)

                z2e = ps.tile([128, FD], f32, tag="pD", name=f"z2e_{e}")
                nc.tensor.matmul(z2e[:], smt["e0"][:], t1eh[:], start=True, stop=False)
                nc.tensor.matmul(z2e[:], smt["e0"][:], t1el[:], start=False, stop=True)
                z2ep = ps.tile([128, FD], f32, tag="pE", name=f"z2ep_{e}")
                nc.tensor.matmul(z2ep[:], smt["e0"][:], a1p[:], start=True, stop=True)
                z2epp = ps.tile([128, FD], f32, tag="pT0", name=f"z2epp_{e}")
                nc.tensor.matmul(z2epp[:], smt["e0"][:], zin[:], start=True, stop=True)
                Dz = ps.tile([128, FD], f32, tag="pT1", name=f"Dz_{e}")
                nc.tensor.matmul(Dz[:], smt["eq"][:], m_[:], start=True, stop=True)
                DyN = ps.tile([128, FD], f32, tag="pB", name=f"DyN_{e}")
                nc.tensor.matmul(DyN[:], smt["ep"][:], m_[:], start=True, stop=True)
                DzpN = ps.tile([128, FD], f32, tag="pC", name=f"DzpN_{e}")
                nc.tensor.matmul(DzpN[:], smt["eq"][:], mpc[:], start=True, stop=True)
                DypN = ps.tile([128, FD], f32, tag="pBC", name=f"DypN_{e}")
                nc.tensor.matmul(DypN[:], smt["ep"][:], mpc[:], start=True, stop=True)
                Dzpp2 = ps.tile([128, FD], f32, tag="pMV", name=f"Dzpp2_{e}")
                nc.tensor.matmul(Dzpp2[:], smt["eq"][:], O2m[:], start=True, stop=True)

                t2e = scr.tile([128, FD], f16, tag="s1_1", name=f"t2e_{e}")
                nc.scalar.activation(t2e[:], z2e[:], AF.Tanh, bias=smt["be2b2"][:])
                s2e = scr.tile([128, FD], f16, tag="tp1f0", name=f"s2e_{e}")
                TT(nc.vector, s2e[:], t2e[:], t2e[:])
                w_ = scr.tile([128, FD], f16, tag="tp1f1", name=f"w_{e}")
                nc.vector.tensor_scalar(w_[:], s2e[:], -1.0, 1.0, ALU.mult, ALU.add)
                z2ep16 = scr.tile([128, FD], f16, tag="z2ep16", name=f"z2ep16_{e}")
                nc.scalar.copy(z2ep16[:], z2ep[:])
                z2ep2 = scr.tile([128, FD], f16, tag="z2ep2", name=f"z2ep2_{e}")
                TT(nc.gpsimd, z2ep2[:], z2ep16[:], z2ep16[:])
                Q1 = scr.tile([128, FD], f16, tag="Q1", name=f"Q1_{e}")
                TT(nc.gpsimd, Q1[:], t2e[:], w_[:])
                wpc = scr.tile([128, FD], f16, tag="wpc", name=f"wpc_{e}")
                TT(nc.vector, wpc[:], Q1[:], z2ep16[:])
                R1 = scr.tile([128, FD], f16, tag="R1", name=f"R1_{e}")
                nc.vector.scalar_tensor_tensor(
                    R1[:], s2e[:], 1.0 / 3.0, w_[:], ALU.subtract, ALU.mult)
                R2f = scr.tile([128, FD], f16, tag="R2f", name=f"R2f_{e}")
                TT(nc.gpsimd, R2f[:], R1[:], z2ep2[:])
                R3f = scr.tile([128, FD], f16, tag="R3f", name=f"R3f_{e}")
                TT(nc.vector, R3f[:], Q1[:], z2epp[:])
                t1m = scr.tile([128, FD], f16, tag="t1m", name=f"t1m_{e}")
                nc.vector.scalar_tensor_tensor(
                    t1m[:], R2f[:], 3.0, R3f[:], ALU.mult, ALU.subtract)
                F1 = scr.tile([128, FD], f16, tag="F1", name=f"F1_{e}")
                TT(nc.vector, F1[:], t1m[:], Dz[:])
                DyNs = scr.tile([128, FD], f16, tag="DyNs", name=f"DyNs_{e}")
                nc.scalar.copy(DyNs[:], DyN[:])
                t2m = scr.tile([128, FD], f16, tag="t2m", name=f"t2m_{e}")
                nc.vector.scalar_tensor_tensor(
                    t2m[:], DzpN[:], 4.0, DyNs[:], ALU.mult, ALU.add)
                F2 = scr.tile([128, FD], f16, tag="F2", name=f"F2_{e}")
                TT(nc.gpsimd, F2[:], wpc[:], t2m[:])
                DypNs = scr.tile([128, FD], f16, tag="DypNs", name=f"DypNs_{e}")
                nc.scalar.copy(DypNs[:], DypN[:])
                t3m = scr.tile([128, FD], f16, tag="t3m", name=f"t3m_{e}")
                TT(nc.vector, t3m[:], Dzpp2[:], DypNs[:], ALU.add)
                F3 = scr.tile([128, FD], f16, tag="F3", name=f"F3_{e}")
                TT(nc.vector, F3[:], w_[:], t3m[:])

                vps = ps.tile([2, FD], f32, tag="pT1", name=f"vps_{e}")
                nc.tensor.matmul(vps[:], smt["v6"][:, 0:2], F1[:], start=True, stop=False)
                nc.tensor.matmul(vps[:], smt["v6"][:, 2:4], F2[:], start=False, stop=False)
                nc.tensor.matmul(vps[:], smt["v6"][:, 4:6], F3[:], start=False, stop=True)
                ot = scr.tile([2, FD], f32, tag="ot", name=f"ot_{e}")
                nc.scalar.copy(ot[:], vps[:])
                nc.sync.dma_start(out_d.ap()[:, e * FD:(e + 1) * FD], ot[:])

    nc.compile()
    return nc


def _get_nc():
    if "nc" not in _CACHE:
        _CACHE["nc"] = _build()
    return _CACHE["nc"]


def kernel(**inputs):
    import concourse.bass_utils as bass_utils

    f = lambda k: np.asarray(inputs[k], np.float32)
    a, x, t = f("a"), f("x"), np.float32(inputs["t"])
    Wb, Wt1, bt1, Wt2, bt2 = f("Wb"), f("Wt1"), f("bt1"), f("Wt2"), f("bt2")
    Wt3, We1, be1, We2, be2, We3 = (
        f("Wt3"), f("We1"), f("be1"), f("We2"), f("be2"), f("We3"))

    h16 = lambda v: np.asarray(v, np.float32).astype(np.float16)
    def pair16(v):
        h = h16(v)
        return h, h16(np.asarray(v, np.float32) - h.astype(np.float32))

    w1 = Wt1[:, 0]
    c1b = (Wt1[:, 1] * t + bt1)[:, None]
    w1h, w1l = pair16(w1)
    w11 = np.stack([w1h, w1h, w1l, w1l])                       # [4,128]
    wt2t = np.ascontiguousarray(Wt2.T)
    mk = lambda M: pair16(M)
    wt2h, wt2l = mk(wt2t)
    w2ah, w2al = mk(wt2t * w1[:, None])
    w2bh, w2bl = mk(wt2t * (-2.0 * w1 ** 2)[:, None])
    w2ch, w2cl = mk(wt2t * (6.0 * w1 ** 3)[:, None])
    wt3h, wt3l = mk(Wt3)

    p, q, v = We1[:, 0], We1[:, 1], We3[0]
    ph, pl = pair16(p)
    qh, ql = pair16(q)
    # mov12 rows: 0 uhA, 1 ulA, 2 uhB, 3 ulB, 4 uxhA, 5 uxlA, 6 uxhB, 7 uxlB,
    #             8 uxxA, 9 uxxB, 10 uxxxA, 11 uxxxB
    A_, B_ = slice(0, 64), slice(64, 128)
    def stat12(rows):
        S = np.zeros((12, 128), np.float16)
        for r, vec, cs in rows:
            S[r, cs] = vec
        return S
    SEH = stat12([(0, ph, A_), (1, ph, A_), (2, ph, B_), (3, ph, B_),
                  (4, qh, A_), (5, qh, A_), (6, qh, B_), (7, qh, B_)])
    SEL = stat12([(0, pl, A_), (2, pl, B_), (4, ql, A_), (6, ql, B_)])
    SPH = stat12([(4, ph, A_), (5, ph, A_), (6, ph, B_), (7, ph, B_),
                  (8, qh, A_), (9, qh, B_)])
    SPL = stat12([(4, pl, A_), (6, pl, B_), (8, ql, A_), (9, ql, B_)])
    SPPH = stat12([(8, ph, A_), (9, ph, B_), (10, qh, A_), (11, qh, B_)])
    SPPL = stat12([(8, pl, A_), (9, pl, B_), (10, ql, A_), (11, ql, B_)])

    blk = lambda M: np.block([[M, np.zeros_like(M)], [np.zeros_like(M), M]])
    We2T = We2.T
    e0 = h16(blk(We2T))
    eq = h16(blk(We2T * q[:, None]))
    ep = h16(blk(We2T * p[:, None]))
    v6 = np.zeros((128, 6), np.float16)
    for i in range(3):
        v6[0:64, 2 * i] = h16(2.0 * v)
        v6[64:128, 2 * i + 1] = h16(2.0 * v)
    sel4m = np.zeros((8, 4), np.float32)
    for j in range(4):
        sel4m[2 * j, j] = 1.0
        sel4m[2 * j + 1, j] = 1.0

    smalls = {
        "w11": w11, "c1b": c1b.astype(np.float32), "bt2b": bt2[:, None].astype(np.float32),
        "wt2h": wt2h, "wt2l": wt2l, "w2ah": w2ah, "w2al": w2al,
        "w2bh": w2bh, "w2bl": w2bl, "w2ch": w2ch, "w2cl": w2cl,
        "wt3h": wt3h, "wt3l": wt3l,
        "SEH": SEH, "SEL": SEL, "SPH": SPH, "SPL": SPL, "SPPH": SPPH, "SPPL": SPPL,
        "e0": e0, "eq": eq, "ep": ep, "v6": v6,
        "be1b2": np.concatenate([be1, be1])[:, None].astype(np.float32),
        "be2b2": np.concatenate([be2, be2])[:, None].astype(np.float32),
        "sel4m": sel4m,
    }
    smalls = {k: np.ascontiguousarray(val) for k, val in smalls.items()}

    in_maps = []
    for c in range(NCORES):
        blk_w = Wb[:, c * KSH:(c + 1) * KSH]                   # [128, 65536]
        tr = blk_w.T.reshape(NKT, 128, 128).transpose(1, 0, 2)  # [k1, kt, p]
        tr = tr.reshape(128, NCHUNK, KTC * 128).transpose(1, 0, 2)
        wsh = np.ascontiguousarray(h16(1024.0 * tr))           # [16,128,4096]
        ash = (a[c * KSH:(c + 1) * KSH] / 1024.0).reshape(NKT, 128).T  # [k1, kt]
        ah, al = pair16(ash)
        a2 = np.ascontiguousarray(np.stack([ah, al], axis=2))  # [128,512,2]
        xs = x[c * NPTS:(c + 1) * NPTS]
        xh, xl = pair16(xs)
        x4 = np.ascontiguousarray(np.stack([xh, xl, xh, xl]))  # [4,4096]
        im = {"w": wsh, "a2": a2, "x4": x4}
        im.update(smalls)
        in_maps.append(im)

    global _last_in_maps
    _last_in_maps = in_maps
    nc = _get_nc()
    res = bass_utils.run_bass_kernel_spmd(nc, in_maps, core_ids=list(range(NCORES)))
    outs = []
    for c in range(NCORES):
        o = res.results[c]["out"]          # [2, NPTS//2]
        outs.append(np.asarray(o).reshape(-1))
    return np.concatenate(outs).astype(np.float32)


# revision 24
# speedup vs baseline: 1.4604x; 1.0662x over previous
"""Bass/Trainium2 kernel for nn_HNO_37065567764989 (self-contained).

Strategy (8 NeuronCores, SPMD):
- Branch matvec b = Wb@a column-sharded 8 ways. Each core streams its 16MB
  shard as fp16 (W scaled by 2^10 to stay normal; a carried as an fp16 hi/lo
  stationary pair, M=8 batched over 4 k-tiles per matmul). 512B AllReduce
  combines partials.
- Nx=32768 points sharded 8 ways (4096/core). Trunk layer-1 overlaps the Wb
  stream; layer-2 (z2/A/B/C, fp16 hi/lo pair stationaries+movings) fills the
  AllReduce latency; EnergyNet runs after, extracting u/u_x/u_xx/u_xxx rows
  with a c=Wt3^T b fp16 stationary and building first-layer preactivations
  from stacked hi/lo row movings (2 matmuls each via host (p,q) stationaries).
- High-sensitivity values flow as fp16 hi/lo pairs (~2^-24); low-sensitivity
  operands are single fp16. All matmuls run at 1 cy/row.
"""
import sys

for _p in ("/opt/trn_rl_repo",):
    if _p not in sys.path:
        sys.path.insert(0, _p)

import numpy as np

MP1, NX, P, HT, HE = 524288, 32768, 128, 128, 64
NCORES = 8
KSH = MP1 // NCORES        # 65536 contraction elems per core
NKT = KSH // 128           # 512 k-tiles
NCHUNK = 16
KTC = NKT // NCHUNK        # 32 k-tiles per chunk
NPTS = NX // NCORES        # 4096 points per core
FD = 512
NTRUNK = NPTS // FD        # 8 trunk tiles
NEN = NTRUNK // 2          # 4 energy tiles (two halves stacked)

_CACHE = {}


def _build():
    import concourse.bacc as bacc
    import concourse.mybir as mybir
    from concourse import tile

    f32 = mybir.dt.float32
    f16 = mybir.dt.float16
    AF = mybir.ActivationFunctionType
    ALU = mybir.AluOpType

    nc = bacc.Bacc("TRN2", target_bir_lowering=False, debug=False,
                   num_devices=NCORES)

    w_d = nc.dram_tensor("w", [NCHUNK, 128, KTC * 128], f16, kind="ExternalInput")
    a_d = nc.dram_tensor("a2", [128, NKT, 2], f16, kind="ExternalInput")
    x_d = nc.dram_tensor("x4", [4, NPTS], f16, kind="ExternalInput")
    sm = {}
    for name, shape, dt in [
        ("w11", [4, 128], f16), ("c1b", [128, 1], f32), ("bt2b", [128, 1], f32),
        ("wt2h", [128, 128], f16), ("wt2l", [128, 128], f16),
        ("w2ah", [128, 128], f16), ("w2al", [128, 128], f16),
        ("w2bh", [128, 128], f16), ("w2bl", [128, 128], f16),
        ("w2ch", [128, 128], f16), ("w2cl", [128, 128], f16),
        ("wt3h", [128, 128], f16), ("wt3l", [128, 128], f16),
        ("SEH", [12, 128], f16), ("SEL", [12, 128], f16),
        ("SPH", [12, 128], f16), ("SPL", [12, 128], f16),
        ("SPPH", [12, 128], f16), ("SPPL", [12, 128], f16),
        ("e0", [128, 128], f16), ("eq", [128, 128], f16), ("ep", [128, 128], f16),
        ("v6", [128, 6], f16),
        ("be1b2", [128, 1], f32), ("be2b2", [128, 1], f32),
        ("sel4m", [8, 4], f32),
    ]:
        sm[name] = nc.dram_tensor(name, shape, dt, kind="ExternalInput")
    out_d = nc.dram_tensor("out", [2, NPTS // 2], f32, kind="ExternalOutput")
    cc_in = nc.dram_tensor("cc_in", [128, 1], f32)
    cc_out = nc.dram_tensor("cc_out", [128, 1], f32, addr_space="Shared")

    def TT(eng, out, i0, i1, op=ALU.mult):
        eng.tensor_tensor(out, i0, i1, op)

    with tile.TileContext(nc) as tc:
        with (
            tc.tile_pool(name="smp", bufs=1) as smp,
            tc.tile_pool(name="persist", bufs=1) as persist,
            tc.tile_pool(name="wpool", bufs=2) as wpool,
            tc.tile_pool(name="scr", bufs=1) as scr,
            tc.tile_pool(name="ps8", bufs=1, space="PSUM") as ps,
        ):
            smt = {}
            for name, h in sm.items():
                t = smp.tile(list(h.shape), h.dtype, name=f"sb_{name}")
                nc.sync.dma_start(t[:], h.ap())
                smt[name] = t
            x4 = smp.tile([4, NPTS], f16, name="x4t")
            nc.sync.dma_start(x4[:], x_d.ap())
            a2 = smp.tile([128, NKT, 2], f16, name="a2t")
            nc.sync.dma_start(a2[:], a_d.ap())
            ones11 = smp.tile([1, 1], f32, name="ones11")
            nc.vector.memset(ones11[:], 1.0)

            # ---- trunk layer-1 z1 matmuls ----
            z1tags = ["pT0", "pT1", "pB", "pC"]
            z1ps = []
            for f in range(NTRUNK):
                cs = slice(f * FD, (f + 1) * FD)
                z1 = ps.tile([128, FD], f32, tag=z1tags[f % 4], name=f"z1_{f}")
                nc.tensor.matmul(z1[:], smt["w11"][:], x4[:, cs], start=True, stop=True)
                z1ps.append(z1)

            # ---- matvec: stream W shard; layer-1 elementwise rides along ----
            l1 = {}
            b8 = ps.tile([8, FD], f32, tag="pMV", name="b8")
            for i in range(NCHUNK):
                wch = wpool.tile([128, KTC * 128], f16, tag="wch", name="wch")
                half = KTC * 64
                nc.sync.dma_start(wch[:, 0:half], w_d.ap()[i][:, 0:half])
                nc.sync.dma_start(wch[:, half:], w_d.ap()[i][:, half:])
                for g in range(KTC // 4):
                    nc.tensor.matmul(
                        b8[:], a2[:, i * KTC + 4 * g:i * KTC + 4 * (g + 1), :],
                        wch[:, g * 512:(g + 1) * 512],
                        start=(i == 0 and g == 0),
                        stop=(i == NCHUNK - 1 and g == KTC // 4 - 1),
                    )
                if i < NTRUNK:
                    f = i
                    z1 = z1ps[f]
                    t1f = scr.tile([128, FD], f32, tag=f"t1f{f % 2}", name=f"t1f_{f}")
                    nc.scalar.activation(t1f[:], z1[:], AF.Tanh, bias=smt["c1b"][:])
                    t1h = persist.tile([128, FD], f16, tag=f"t1h_{f}", name=f"t1h_{f}")
                    nc.scalar.copy(t1h[:], t1f[:])
                    t1l = persist.tile([128, FD], f16, tag=f"t1l_{f}", name=f"t1l_{f}")
                    TT(nc.vector, t1l[:], t1f[:], t1h[:], ALU.subtract)
                    s1 = scr.tile([128, FD], f32, tag=f"s1_{f % 2}", name=f"s1_{f}")
                    nc.scalar.square(s1[:], t1f[:])
                    tp1f = scr.tile([128, FD], f32, tag=f"tp1f{f % 2}", name=f"tp1f_{f}")
                    nc.vector.tensor_scalar(tp1f[:], s1[:], -1.0, 1.0, ALU.mult, ALU.add)
                    tp1h = persist.tile([128, FD], f16, tag=f"tp1h_{f}", name=f"tp1h_{f}")
                    nc.scalar.copy(tp1h[:], tp1f[:])
                    tp1l = persist.tile([128, FD], f16, tag=f"tp1l_{f}", name=f"tp1l_{f}")
                    TT(nc.vector, tp1l[:], tp1f[:], tp1h[:], ALU.subtract)
                    g2m = persist.tile([128, FD], f16, tag=f"g2m_{f}", name=f"g2m_{f}")
                    TT(nc.vector, g2m[:], t1f[:], tp1f[:])
                    g3m = persist.tile([128, FD], f16, tag=f"g3m_{f}", name=f"g3m_{f}")
                    nc.vector.scalar_tensor_tensor(
                        g3m[:], s1[:], 1.0 / 3.0, tp1f[:], ALU.subtract, ALU.mult)
                    l1[f] = (t1h, t1l, tp1h, tp1l, g2m, g3m)

            # ---- local reduce + AllReduce ----
            b8sb = smp.tile([8, FD], f32, name="b8sb")
            nc.scalar.copy(b8sb[:], b8[:])
            bcol = ps.tile([128, 1], f32, tag="pBC", name="bcol")
            for j in range(4):
                nc.tensor.matmul(bcol[:], b8sb[:, j * 128:(j + 1) * 128],
                                 smt["sel4m"][:, j:j + 1],
                                 start=(j == 0), stop=(j == 3))
            b_loc = smp.tile([128, 1], f32, name="bloc")
            nc.scalar.copy(b_loc[:], bcol[:])
            nc.sync.dma_start(cc_in.ap(), b_loc[:])
            nc.gpsimd.collective_compute(
                "AllReduce", ALU.add,
                replica_groups=[list(range(NCORES))],
                ins=[cc_in.ap()], outs=[cc_out.ap()],
            )
            b_ar = smp.tile([128, 1], f32, name="bar")
            nc.sync.dma_start(b_ar[:], cc_out.ap())

            # ---- trunk layer-2 wave (fills AllReduce latency) ----
            # stage-2 outputs for trunk tiles f and f+4 share one [128,1024]
            # tile (halves side by side in the free dim) so energy extracts
            # can read both halves of an energy tile from one tile family.
            sh = {}
            for f in range(NTRUNK):
                j, off = f % NEN, (f // NEN) * FD
                t1h, t1l, tp1h, tp1l, g2m, g3m = l1[f]
                if f < NEN:
                    sh[j] = tuple(
                        persist.tile([128, 2 * FD], f16, tag=f"sh{nm}_{j}",
                                     name=f"sh_{nm}_{j}")
                        for nm in ("t2h", "t2l", "P1h", "P1l", "ux2", "ux3"))
                t2h_s, t2l_s, P1h_s, P1l_s, ux2_s, ux3_s = sh[j]
                osl = slice(off, off + FD)
                z2 = ps.tile([128, FD], f32, tag="pT0", name=f"z2_{f}")
                nc.tensor.matmul(z2[:], smt["wt2h"][:], t1h[:], start=True, stop=False)
                nc.tensor.matmul(z2[:], smt["wt2h"][:], t1l[:], start=False, stop=False)
                nc.tensor.matmul(z2[:], smt["wt2l"][:], t1h[:], start=False, stop=True)
                A = ps.tile([128, FD], f32, tag="pT1", name=f"A_{f}")
                nc.tensor.matmul(A[:], smt["w2ah"][:], tp1h[:], start=True, stop=False)
                nc.tensor.matmul(A[:], smt["w2ah"][:], tp1l[:], start=False, stop=False)
                nc.tensor.matmul(A[:], smt["w2al"][:], tp1h[:], start=False, stop=True)
                B = ps.tile([128, FD], f32, tag="pB", name=f"B_{f}")
                nc.tensor.matmul(B[:], smt["w2bh"][:], g2m[:], start=True, stop=False)
                nc.tensor.matmul(B[:], smt["w2bl"][:], g2m[:], start=False, stop=True)
                C = ps.tile([128, FD], f32, tag="pC", name=f"C_{f}")
                nc.tensor.matmul(C[:], smt["w2ch"][:], g3m[:], start=True, stop=False)
                nc.tensor.matmul(C[:], smt["w2cl"][:], g3m[:], start=False, stop=True)

                t2f = scr.tile([128, FD], f32, tag="t2f", name=f"t2f_{f}")
                nc.scalar.activation(t2f[:], z2[:], AF.Tanh, bias=smt["bt2b"][:])
                nc.scalar.copy(t2h_s[:, osl], t2f[:])
                TT(nc.vector, t2l_s[:, osl], t2f[:], t2h_s[:, osl], ALU.subtract)
                s2 = scr.tile([128, FD], f32, tag="s2", name=f"s2_{f}")
                nc.scalar.square(s2[:], t2f[:])
                tp2 = scr.tile([128, FD], f32, tag="tp2", name=f"tp2_{f}")
                nc.vector.tensor_scalar(tp2[:], s2[:], -1.0, 1.0, ALU.mult, ALU.add)
                A2 = scr.tile([128, FD], f32, tag="A2", name=f"A2_{f}")
                nc.scalar.square(A2[:], A[:])
                P1f = scr.tile([128, FD], f32, tag="P1f", name=f"P1f_{f}")
                TT(nc.vector, P1f[:], tp2[:], A[:])
                nc.scalar.copy(P1h_s[:, osl], P1f[:])
                TT(nc.vector, P1l_s[:, osl], P1f[:], P1h_s[:, osl], ALU.subtract)
                M4 = scr.tile([128, FD], f32, tag="M4", name=f"M4_{f}")
                TT(nc.gpsimd, M4[:], tp2[:], A2[:])
                M5 = scr.tile([128, FD], f32, tag="M5", name=f"M5_{f}")
                TT(nc.gpsimd, M5[:], t2f[:], M4[:])
                M6 = scr.tile([128, FD], f32, tag="M6", name=f"M6_{f}")
                TT(nc.vector, M6[:], tp2[:], B[:])
                nc.vector.scalar_tensor_tensor(
                    ux2_s[:, osl], M5[:], -2.0, M6[:], ALU.mult, ALU.add)
                A3 = scr.tile([128, FD], f32, tag="A3", name=f"A3_{f}")
                TT(nc.vector, A3[:], A2[:], A[:])
                V = scr.tile([128, FD], f32, tag="V", name=f"V_{f}")
                nc.vector.scalar_tensor_tensor(
                    V[:], s2[:], 1.0 / 3.0, tp2[:], ALU.subtract, ALU.mult)
                M1 = scr.tile([128, FD], f32, tag="M1", name=f"M1_{f}")
                TT(nc.gpsimd, M1[:], V[:], A3[:])
                W1 = scr.tile([128, FD], f32, tag="W1", name=f"W1_{f}")
                TT(nc.vector, W1[:], P1f[:], B[:])
                M2 = scr.tile([128, FD], f32, tag="M2", name=f"M2_{f}")
                TT(nc.gpsimd, M2[:], t2f[:], W1[:])
                M3 = scr.tile([128, FD], f32, tag="M3", name=f"M3_{f}")
                TT(nc.vector, M3[:], tp2[:], C[:])
                D1 = scr.tile([128, FD], f32, tag="D1", name=f"D1_{f}")
                TT(nc.gpsimd, D1[:], M1[:], M2[:], ALU.subtract)
                nc.vector.scalar_tensor_tensor(
                    ux3_s[:, osl], D1[:], 6.0, M3[:], ALU.mult, ALU.add)

            # ---- c = Wt3^T b (fp16 single stationary) ----
            b16 = smp.tile([128, 1], f16, name="b16")
            nc.scalar.copy(b16[:], b_ar[:])
            c0p = ps.tile([1, 128], f32, tag="pT0", name="c0p")
            nc.tensor.matmul(c0p[:], b16[:], smt["wt3h"][:], start=True, stop=False)
            nc.tensor.matmul(c0p[:], b16[:], smt["wt3l"][:], start=False, stop=True)
            c0 = smp.tile([1, 128], f32, name="c0")
            nc.scalar.copy(c0[:], c0p[:])
            ct = ps.tile([128, 1], f32, tag="pT1", name="ct")
            nc.tensor.matmul(ct[:], c0[:], ones11[:], start=True, stop=True)
            c16 = smp.tile([128, 1], f16, name="c16")
            nc.scalar.copy(c16[:], ct[:])

            # ---- energy phase: hoisted extracts for all tiles ----
            exttags = ["pT0", "pT1"]
            mov12s = {}
            ti = 0
            for e in range(NEN):
                t2h_s, t2l_s, P1h_s, P1l_s, ux2_s, ux3_s = sh[e]
                mov12 = scr.tile([12, FD], f16, tag=f"mv12_{e}", name=f"mv12_{e}")
                mov12s[e] = mov12
                for qi, movs in enumerate(((t2h_s, t2l_s), (P1h_s, P1l_s),
                                           (ux2_s,), (ux3_s,))):
                    hlw = 2 * FD if qi < 2 else FD
                    hlab = wpool.tile([1, 2 * hlw], f16, tag="wch",
                                      name=f"hlab{e}_{qi}")
                    for hx in range(2):
                        osl = slice(hx * FD, (hx + 1) * FD)
                        uq = ps.tile([1, FD], f32, tag=exttags[ti % 2],
                                     name=f"uq{e}_{qi}_{hx}")
                        ti += 1
                        for mi, mv in enumerate(movs):
                            nc.tensor.matmul(uq[:], c16[:], mv[:, osl],
                                             start=(mi == 0),
                                             stop=(mi == len(movs) - 1))
                        if qi < 2:
                            nc.scalar.copy(hlab[:, hx * hlw:hx * hlw + FD], uq[:])
                            TT(nc.vector, hlab[:, hx * hlw + FD:(hx + 1) * hlw],
                               uq[:], hlab[:, hx * hlw:hx * hlw + FD], ALU.subtract)
                        else:
                            nc.scalar.copy(hlab[:, hx * FD:(hx + 1) * FD], uq[:])
                    # rows: qi=0 -> 0:4 (uhA,ulA,uhB,ulB); qi=1 -> 4:8;
                    # qi=2 -> 8:10; qi=3 -> 10:12
                    r0 = qi * 4 if qi < 2 else 4 + qi * 2
                    nr = 4 if qi < 2 else 2
                    nc.sync.dma_start(mov12[r0:r0 + nr, :], hlab[:])

            for e in range(NEN):
                mov12 = mov12s[e]
                trio = [["pB", "pC", "pBC"], ["pD", "pE", "pMV"]][e % 2]
                dzt, dyt = ("pT0", "pT1") if e % 2 == 0 else ("pT1", "pT0")
                z1e = ps.tile([128, FD], f32, tag=trio[0], name=f"z1e_{e}")
                nc.tensor.matmul(z1e[:], smt["SEH"][:], mov12[:], start=True, stop=False)
                nc.tensor.matmul(z1e[:], smt["SEL"][:], mov12[:], start=False, stop=True)
                z1p = ps.tile([128, FD], f32, tag=trio[1], name=f"z1p_{e}")
                nc.tensor.matmul(z1p[:], smt["SPH"][:], mov12[:], start=True, stop=False)
                nc.tensor.matmul(z1p[:], smt["SPL"][:], mov12[:], start=False, stop=True)
                z1pp = ps.tile([128, FD], f32, tag=trio[2], name=f"z1pp_{e}")
                nc.tensor.matmul(z1pp[:], smt["SPPH"][:], mov12[:], start=True, stop=False)
                nc.tensor.matmul(z1pp[:], smt["SPPL"][:], mov12[:], start=False, stop=True)

                t1ef = scr.tile([128, FD], f32, tag="t2f", name=f"t1ef_{e}")
                nc.scalar.activation(t1ef[:], z1e[:], AF.Tanh, bias=smt["be1b2"][:])
                t1eh = scr.tile([128, FD], f16, tag="s2", name=f"t1eh_{e}")
                nc.scalar.copy(t1eh[:], t1ef[:])
                t1el = scr.tile([128, FD], f16, tag="tp2", name=f"t1el_{e}")
                TT(nc.vector, t1el[:], t1ef[:], t1eh[:], ALU.subtract)
                z1psb = scr.tile([128, FD], f16, tag="A2", name=f"z1psb_{e}")
                nc.scalar.copy(z1psb[:], z1p[:])
                z1ppsb = scr.tile([128, FD], f16, tag="P1f", name=f"z1ppsb_{e}")
                nc.scalar.copy(z1ppsb[:], z1pp[:])
                s1e = scr.tile([128, FD], f16, tag="M4", name=f"s1e_{e}")
                nc.scalar.square(s1e[:], t1ef[:])
                m_ = scr.tile([128, FD], f16, tag="M5", name=f"m_{e}")
                nc.vector.tensor_scalar(m_[:], s1e[:], -1.0, 1.0, ALU.mult, ALU.add)
                z1p2 = scr.tile([128, FD], f16, tag="M6", name=f"z1p2_{e}")
                TT(nc.gpsimd, z1p2[:], z1psb[:], z1psb[:])
                N1 = scr.tile([128, FD], f16, tag="A3", name=f"N1_{e}")
                TT(nc.gpsimd, N1[:], t1ef[:], m_[:])
                a1p = scr.tile([128, FD], f16, tag="V", name=f"a1p_{e}")
                TT(nc.vector, a1p[:], m_[:], z1psb[:])
                N2 = scr.tile([128, FD], f16, tag="M1", name=f"N2_{e}")
                TT(nc.gpsimd, N2[:], N1[:], z1p2[:])
                N3 = scr.tile([128, FD], f16, tag="W1", name=f"N3_{e}")
                TT(nc.vector, N3[:], m_[:], z1ppsb[:])
                zin = scr.tile([128, FD], f16, tag="M2", name=f"zin_{e}")
                nc.vector.scalar_tensor_tensor(
                    zin[:], N2[:], -2.0, N3[:], ALU.mult, ALU.add)
                mpc = scr.tile([128, FD], f16, tag="M3", name=f"mpc_{e}")
                TT(nc.vector, mpc[:], N1[:], z1psb[:])
                O1 = scr.tile([128, FD], f16, tag="D1", name=f"O1_{e}")
                nc.vector.scalar_tensor_tensor(
                    O1[:], s1e[:], 1.0 / 3.0, m_[:], ALU.subtract, ALU.mult)
                O2f = scr.tile([128, FD], f16, tag="t1f0", name=f"O2f_{e}")
                TT(nc.gpsimd, O2f[:], O1[:], z1p2[:])
                O3f = scr.tile([128, FD], f16, tag="t1f1", name=f"O3f_{e}")
                TT(nc.vector, O3f[:], N1[:], z1ppsb[:])
                O2m = scr.tile([128, FD], f16, tag="s1_0", name=f"O2m_{e}")
                nc.vector.scalar_tensor_tensor(
                    O2m[:], O2f[:], 3.0, O3f[:], ALU.mult, ALU.subtract)

                z2e = ps.tile([128, FD], f32, tag=trio[0], name=f"z2e_{e}")
                nc.tensor.matmul(z2e[:], smt["e0"][:], t1eh[:], start=True, stop=False)
                nc.tensor.matmul(z2e[:], smt["e0"][:], t1el[:], start=False, stop=True)
                z2ep = ps.tile([128, FD], f32, tag=trio[1], name=f"z2ep_{e}")
                nc.tensor.matmul(z2ep[:], smt["e0"][:], a1p[:], start=True, stop=True)
                z2epp = ps.tile([128, FD], f32, tag=trio[2], name=f"z2epp_{e}")
                nc.tensor.matmul(z2epp[:], smt["e0"][:], zin[:], start=True, stop=True)
                Dz = ps.tile([128, FD], f32, tag=dzt, name=f"Dz_{e}")
                nc.tensor.matmul(Dz[:], smt["eq"][:], m_[:], start=True, stop=True)
                DyN = ps.tile([128, FD], f32, tag=dyt, name=f"DyN_{e}")
                nc.tensor.matmul(DyN[:], smt["ep"][:], m_[:], start=True, stop=True)
                DzpN = ps.tile([128, FD], f32, tag=trio[0], name=f"DzpN_{e}")
                nc.tensor.matmul(DzpN[:], smt["eq"][:], mpc[:], start=True, stop=True)
                DypN = ps.tile([128, FD], f32, tag=trio[1], name=f"DypN_{e}")
                nc.tensor.matmul(DypN[:], smt["ep"][:], mpc[:], start=True, stop=True)
                Dzpp2 = ps.tile([128, FD], f32, tag=trio[2], name=f"Dzpp2_{e}")
                nc.tensor.matmul(Dzpp2[:], smt["eq"][:], O2m[:], start=True, stop=True)

                t2e = scr.tile([128, FD], f16, tag="s1_1", name=f"t2e_{e}")
                nc.scalar.activation(t2e[:], z2e[:], AF.Tanh, bias=smt["be2b2"][:])
                s2e = scr.tile([128, FD], f16, tag="tp1f0", name=f"s2e_{e}")
                TT(nc.vector, s2e[:], t2e[:], t2e[:])
                w_ = scr.tile([128, FD], f16, tag="tp1f1", name=f"w_{e}")
                nc.vector.tensor_scalar(w_[:], s2e[:], -1.0, 1.0, ALU.mult, ALU.add)
                z2ep16 = scr.tile([128, FD], f16, tag="z2ep16", name=f"z2ep16_{e}")
                nc.scalar.copy(z2ep16[:], z2ep[:])
                z2ep2 = scr.tile([128, FD], f16, tag="z2ep2", name=f"z2ep2_{e}")
                TT(nc.gpsimd, z2ep2[:], z2ep16[:], z2ep16[:])
                Q1 = scr.tile([128, FD], f16, tag="Q1", name=f"Q1_{e}")
                TT(nc.gpsimd, Q1[:], t2e[:], w_[:])
                wpc = scr.tile([128, FD], f16, tag="wpc", name=f"wpc_{e}")
                TT(nc.vector, wpc[:], Q1[:], z2ep16[:])
                R1 = scr.tile([128, FD], f16, tag="R1", name=f"R1_{e}")
                nc.vector.scalar_tensor_tensor(
                    R1[:], s2e[:], 1.0 / 3.0, w_[:], ALU.subtract, ALU.mult)
                R2f = scr.tile([128, FD], f16, tag="R2f", name=f"R2f_{e}")
                TT(nc.gpsimd, R2f[:], R1[:], z2ep2[:])
                R3f = scr.tile([128, FD], f16, tag="R3f", name=f"R3f_{e}")
                TT(nc.vector, R3f[:], Q1[:], z2epp[:])
                t1m = scr.tile([128, FD], f16, tag="t1m", name=f"t1m_{e}")
                nc.vector.scalar_tensor_tensor(
                    t1m[:], R2f[:], 3.0, R3f[:], ALU.mult, ALU.subtract)
                F1 = scr.tile([128, FD], f16, tag="F1", name=f"F1_{e}")
                TT(nc.vector, F1[:], t1m[:], Dz[:])
                DyNs = scr.tile([128, FD], f16, tag="DyNs", name=f"DyNs_{e}")
                nc.scalar.copy(DyNs[:], DyN[:])
                t2m = scr.tile([128, FD], f16, tag="t2m", name=f"t2m_{e}")
                nc.vector.scalar_tensor_tensor(
                    t2m[:], DzpN[:], 4.0, DyNs[:], ALU.mult, ALU.add)
                F2 = scr.tile([128, FD], f16, tag="F2", name=f"F2_{e}")
                TT(nc.gpsimd, F2[:], wpc[:], t2m[:])
                DypNs = scr.tile([128, FD], f16, tag="DypNs", name=f"DypNs_{e}")
                nc.scalar.copy(DypNs[:], DypN[:])
                t3m = scr.tile([128, FD], f16, tag="t3m", name=f"t3m_{e}")
                TT(nc.vector, t3m[:], Dzpp2[:], DypNs[:], ALU.add)
                F3 = scr.tile([128, FD], f16, tag="F3", name=f"F3_{e}")
                TT(nc.vector, F3[:], w_[:], t3m[:])

                vps = ps.tile([2, FD], f32, tag=trio[1], name=f"vps_{e}")
                nc.tensor.matmul(vps[:], smt["v6"][:, 0:2], F1[:], start=True, stop=False)
                nc.tensor.matmul(vps[:], smt["v6"][:, 2:4], F2[:], start=False, stop=False)
                nc.tensor.matmul(vps[:], smt["v6"][:, 4:6], F3[:], start=False, stop=True)
                ot = scr.tile([2, FD], f32, tag="ot", name=f"ot_{e}")
                nc.scalar.copy(ot[:], vps[:])
                nc.sync.dma_start(out_d.ap()[:, e * FD:(e + 1) * FD], ot[:])

    nc.compile()
    return nc


def _get_nc():
    if "nc" not in _CACHE:
        _CACHE["nc"] = _build()
    return _CACHE["nc"]


def kernel(**inputs):
    import concourse.bass_utils as bass_utils

    f = lambda k: np.asarray(inputs[k], np.float32)
    a, x, t = f("a"), f("x"), np.float32(inputs["t"])
    Wb, Wt1, bt1, Wt2, bt2 = f("Wb"), f("Wt1"), f("bt1"), f("Wt2"), f("bt2")
    Wt3, We1, be1, We2, be2, We3 = (
        f("Wt3"), f("We1"), f("be1"), f("We2"), f("be2"), f("We3"))

    h16 = lambda v: np.asarray(v, np.float32).astype(np.float16)
    def pair16(v):
        h = h16(v)
        return h, h16(np.asarray(v, np.float32) - h.astype(np.float32))

    w1 = Wt1[:, 0]
    c1b = (Wt1[:, 1] * t + bt1)[:, None]
    w1h, w1l = pair16(w1)
    w11 = np.stack([w1h, w1h, w1l, w1l])                       # [4,128]
    wt2t = np.ascontiguousarray(Wt2.T)
    mk = lambda M: pair16(M)
    wt2h, wt2l = mk(wt2t)
    w2ah, w2al = mk(wt2t * w1[:, None])
    w2bh, w2bl = mk(wt2t * (-2.0 * w1 ** 2)[:, None])
    w2ch, w2cl = mk(wt2t * (6.0 * w1 ** 3)[:, None])
    wt3h, wt3l = mk(Wt3)

    p, q, v = We1[:, 0], We1[:, 1], We3[0]
    ph, pl = pair16(p)
    qh, ql = pair16(q)
    # mov12 rows: 0 uhA, 1 ulA, 2 uhB, 3 ulB, 4 uxhA, 5 uxlA, 6 uxhB, 7 uxlB,
    #             8 uxxA, 9 uxxB, 10 uxxxA, 11 uxxxB
    A_, B_ = slice(0, 64), slice(64, 128)
    def stat12(rows):
        S = np.zeros((12, 128), np.float16)
        for r, vec, cs in rows:
            S[r, cs] = vec
        return S
    SEH = stat12([(0, ph, A_), (1, ph, A_), (2, ph, B_), (3, ph, B_),
                  (4, qh, A_), (5, qh, A_), (6, qh, B_), (7, qh, B_)])
    SEL = stat12([(0, pl, A_), (2, pl, B_), (4, ql, A_), (6, ql, B_)])
    SPH = stat12([(4, ph, A_), (5, ph, A_), (6, ph, B_), (7, ph, B_),
                  (8, qh, A_), (9, qh, B_)])
    SPL = stat12([(4, pl, A_), (6, pl, B_), (8, ql, A_), (9, ql, B_)])
    SPPH = stat12([(8, ph, A_), (9, ph, B_), (10, qh, A_), (11, qh, B_)])
    SPPL = stat12([(8, pl, A_), (9, pl, B_), (10, ql, A_), (11, ql, B_)])

    blk = lambda M: np.block([[M, np.zeros_like(M)], [np.zeros_like(M), M]])
    We2T = We2.T
    e0 = h16(blk(We2T))
    eq = h16(blk(We2T * q[:, None]))
    ep = h16(blk(We2T * p[:, None]))
    v6 = np.zeros((128, 6), np.float16)
    for i in range(3):
        v6[0:64, 2 * i] = h16(2.0 * v)
        v6[64:128, 2 * i + 1] = h16(2.0 * v)
    sel4m = np.zeros((8, 4), np.float32)
    for j in range(4):
        sel4m[2 * j, j] = 1.0
        sel4m[2 * j + 1, j] = 1.0

    smalls = {
        "w11": w11, "c1b": c1b.astype(np.float32), "bt2b": bt2[:, None].astype(np.float32),
        "wt2h": wt2h, "wt2l": wt2l, "w2ah": w2ah, "w2al": w2al,
        "w2bh": w2bh, "w2bl": w2bl, "w2ch": w2ch, "w2cl": w2cl,
        "wt3h": wt3h, "wt3l": wt3l,
        "SEH": SEH, "SEL": SEL, "SPH": SPH, "SPL": SPL, "SPPH": SPPH, "SPPL": SPPL,
        "e0": e0, "eq": eq, "ep": ep, "v6": v6,
        "be1b2": np.concatenate([be1, be1])[:, None].astype(np.float32),
        "be2b2": np.concatenate([be2, be2])[:, None].astype(np.float32),
        "sel4m": sel4m,
    }
    smalls = {k: np.ascontiguousarray(val) for k, val in smalls.items()}

    in_maps = []
    for c in range(NCORES):
        blk_w = Wb[:, c * KSH:(c + 1) * KSH]                   # [128, 65536]
        tr = blk_w.T.reshape(NKT, 128, 128).transpose(1, 0, 2)  # [k1, kt, p]
        tr = tr.reshape(128, NCHUNK, KTC * 128).transpose(1, 0, 2)
        wsh = np.ascontiguousarray(h16(1024.0 * tr))           # [16,128,4096]
        ash = (a[c * KSH:(c + 1) * KSH] / 1024.0).reshape(NKT, 128).T  # [k1, kt]
        ah, al = pair16(ash)
        a2 = np.ascontiguousarray(np.stack([ah, al], axis=2))  # [128,512,2]
        xs = x[c * NPTS:(c + 1) * NPTS]
        xh, xl = pair16(xs)
        x4 = np.ascontiguousarray(np.stack([xh, xl, xh, xl]))  # [4,4096]
        im = {"w": wsh, "a2": a2, "x4": x4}
        im.update(smalls)
        in_maps.append(im)

    global _last_in_maps
    _last_in_maps = in_maps
    nc = _get_nc()
    res = bass_utils.run_bass_kernel_spmd(nc, in_maps, core_ids=list(range(NCORES)))
    outs = []
    for c in range(NCORES):
        o = res.results[c]["out"]          # [2, NPTS//2]
        outs.append(np.asarray(o).reshape(-1))
    return np.concatenate(outs).astype(np.float32)
